# revision 1
# baseline (speedup 1.0000x reference)
"""Trainium2 Bass kernel for the binarized CNN (nn_CNN_binary_55001351193058).

Strategy (pure data-parallel over 8 NeuronCores, batch-sharded):
  - Layer 1 (real-valued conv, stride 2, k=9): dense banded matmul over the
    full input width (K=128) in float32r on the PE, producing a duplicated
    row layout that IS conv2's im2col.  x is transposed on-chip via the PE.
  - maxpool+binarize are folded into the PSUM evictions:
      pool = DVE tensor_tensor(max) over the even/odd matmul tiles,
      binarize = one DVE tensor_scalar (>= theta, -0.5) or ACT Sign(+bias).
  - Layers 2-4 + fc: exact small-integer arithmetic in bf16 matmuls
    (binarized +-1/+-0.5 activations, +-1 weights, fp32 PSUM accumulation),
    BatchNorm+Hardtanh+binarize folded into per-channel thresholds computed
    on the host in float64.  Bit-exact vs the fp32 reference except for
    conv1 accumulation-order effects.
All shapes/sharding hardcoded for B=8192, 8 cores, 1024 samples/core.
"""

import numpy as np
import ml_dtypes

import concourse.bass as bass
import concourse.mybir as mybir
import concourse.tile as tile
from concourse import bacc
from concourse.bass_utils import run_bass_kernel_spmd

F32 = mybir.dt.float32
F32R = mybir.dt.float32r
BF16 = mybir.dt.bfloat16
AF = mybir.ActivationFunctionType
ALU = mybir.AluOpType

B_TOTAL = 8192
N_CORES = 8
B_CORE = B_TOTAL // N_CORES          # 1024
NB = 64                              # samples per chunk
N_CHUNKS = B_CORE // NB              # 16
BH = NB * 6                          # 384 (b,h) columns per chunk
EPS = 1e-5

bf16 = ml_dtypes.bfloat16


# ----------------------------------------------------------------------------
# Host-side weight preparation (all in float64 where it matters)
# ----------------------------------------------------------------------------

def _sgn(w):
    return np.where(w >= 0, 1.0, -1.0)


def _threshold(g, be, m, v, bias):
    inv = g.astype(np.float64) / np.sqrt(v.astype(np.float64) + EPS)
    assert (inv > 0).all(), "BN scale must be positive for threshold folding"
    sh = be.astype(np.float64) - m.astype(np.float64) * inv
    return -bias.astype(np.float64) - sh / inv


def _check_margin(th, grid_step, name):
    # distance of each threshold from the reachable z-grid (multiples of
    # grid_step); if the reference's fp32 rounding could flip a sign the
    # margin would have to be ~1e-6 -- assert far above that.
    d = np.abs(th / grid_step - np.round(th / grid_step)) * grid_step
    if d.min() < 1e-4:
        raise AssertionError(f"threshold margin too small for {name}: {d.min()}")


def prepare_host_tensors(w1, b1, w2, b2, w3, b3, w4, b4,
                         g1, be1, m1, v1, g2, be2, m2, v2,
                         g3, be3, m3, v3, g4, be4, m4, v4, wf, bf):
    t1 = _threshold(g1, be1, m1, v1, b1)       # [32]
    t2 = _threshold(g2, be2, m2, v2, b2)       # [64]
    t3 = _threshold(g3, be3, m3, v3, b3)       # [128]
    t4 = _threshold(g4, be4, m4, v4, b4)       # [128]
    _check_margin(t2 / 2.0, 0.5, "t2")         # z2 on 0.5-grid (s1 = +-0.5)
    _check_margin(t3, 2.0, "t3")               # z3 even ints
    _check_margin(t4 / 2.0, 1.0, "t4")         # z4 ints (s3 = +-0.5)

    s1 = _sgn(w1)[:, 0, 0, :].astype(np.float32)        # [32, 9]
    s2 = _sgn(w2)[:, :, 0, :].astype(np.float32)        # [64, 32, 3]
    s3 = _sgn(w3)[:, :, 0, :].astype(np.float32)        # [128, 64, 3]
    s4 = _sgn(w4)[:, :, :, 0].astype(np.float32)        # [128, 128, 6]
    sf = _sgn(wf).astype(np.float32)                    # [10, 2048]

    # conv1 weights: 32 M-tiles (16 u-tiles x even/odd), lhsT layout [w, row]
    # row = p*32 + ci holds y1[ci, wy] with wy = 2*(2u-1+p) + half
    A1 = np.zeros((32, 128, 128), np.float32)
    for mt in range(32):
        u, half = mt // 2, mt % 2
        for p in range(4):
            w1i = 2 * u - 1 + p
            if not (0 <= w1i < 32):
                continue
            wy = 2 * w1i + half
            for k in range(9):
                wx = 2 * wy + k - 4
                if 0 <= wx < 128:
                    A1[mt, wx, p * 32:(p + 1) * 32] = s1[:, k]
    A1 = A1.reshape(32 * 128, 128)  # stacked on free dim? no: [mt*? ] see below
    # store as [128, 32*128]: partitions = w, free = (mt, row)
    A1 = A1.reshape(32, 128, 128).transpose(1, 0, 2).reshape(128, 32 * 128)

    # conv2 pair-im2col weights: lhsT [(p,ci)=128, (op,co)=128]
    W2p = np.zeros((128, 128), np.float32)
    for p in range(4):
        for op in range(2):
            k = p - op
            if 0 <= k <= 2:
                # rows p*32+ci, cols op*64+co = sigma2[co, ci, k]
                W2p[p * 32:(p + 1) * 32, op * 64:(op + 1) * 64] = s2[:, :, k].T
    # conv3 weights. Q rows: [even(w=2u): co 0..63 | odd: co 0..63]
    # tile op3=0 (w=2u):  pass a (K=128, rhs Q[:,u+1]): even->k1, odd->k2
    #                     pass b (K=64,  rhs Q[64:,u]):  odd[u-1]->k0
    # tile op3=1 (w=2u+1):pass a: even->k0, odd->k1
    #                     pass b (K=64, rhs Q[0:64,u+2]): even[u+1]->k2
    W3a = np.zeros((128, 256), np.float32)
    W3a[0:64, 0:128] = s3[:, :, 1].T
    W3a[64:128, 0:128] = s3[:, :, 2].T
    W3a[0:64, 128:256] = s3[:, :, 0].T
    W3a[64:128, 128:256] = s3[:, :, 1].T
    # K=64 passes: base partitions must match the rhs slice of Q, so store
    # the odd-pass weights in rows 64-127 and the even-pass in rows 0-63.
    W3b = np.zeros((128, 256), np.float32)
    W3b[64:128, 0:128] = s3[:, :, 0].T     # rhs Q[64:128, u]   (odd[u-1], k=0)
    W3b[0:64, 128:256] = s3[:, :, 2].T     # rhs Q[0:64, u+2]   (even[u+1], k=2)

    W4t = s4.transpose(2, 1, 0).reshape(6, 128, 128)    # [h][ci, co]
    W4t = W4t.transpose(1, 0, 2).reshape(128, 6 * 128)  # [ci, (h,co)]

    Wf = sf.reshape(10, 128, 16)                         # [j, co, w]
    Wf = Wf.transpose(1, 2, 0).reshape(128, 16 * 10)     # [co, (w,j)]
    Wf = Wf.reshape(128, 16, 10).transpose(0, 1, 2)      # keep [co][w][j]
    Wf = Wf.reshape(128, 160)

    th1 = np.tile(t1, 4).astype(np.float32).reshape(128, 1)          # rows (p,ci)
    bias2 = (-t2 / 2.0).astype(np.float32)
    bias2 = np.concatenate([bias2, bias2]).reshape(128, 1)           # (op,co)
    # th3 applies AFTER pooling (pool over op3 pairs) -> per co3 only
    th3 = t3.astype(np.float32).reshape(128, 1)
    bias4 = (-t4 / 2.0).astype(np.float32).reshape(128, 1)
    bfv = bf.astype(np.float32).reshape(10, 1)
    ident = np.eye(128, dtype=np.float32)

    return dict(
        A1=A1.astype(np.float32), A1bf=A1.astype(bf16),
        W2p=W2p.astype(bf16), W3a=W3a.astype(bf16), W3b=W3b.astype(bf16),
        W4t=W4t.astype(bf16), Wf=Wf.astype(bf16),
        th1=th1, bias2=bias2, th3=th3, bias4=bias4, bfv=bfv, ident=ident,
    )


# ----------------------------------------------------------------------------
# Bass program (identical SPMD program for each core)
# ----------------------------------------------------------------------------

def build_program():
    nc = bacc.Bacc("TRN2", target_bir_lowering=False, debug=False)

    x_d = nc.dram_tensor("x", [B_CORE, 6, 128], F32, kind="ExternalInput").ap()
    A1_d = nc.dram_tensor("A1", [128, 32 * 128], F32R, kind="ExternalInput").ap()
    A1b_d = nc.dram_tensor("A1bf", [128, 32 * 128], BF16, kind="ExternalInput").ap()
    W2_d = nc.dram_tensor("W2p", [128, 128], BF16, kind="ExternalInput").ap()
    W3a_d = nc.dram_tensor("W3a", [128, 256], BF16, kind="ExternalInput").ap()
    W3b_d = nc.dram_tensor("W3b", [128, 256], BF16, kind="ExternalInput").ap()
    W4_d = nc.dram_tensor("W4t", [128, 6 * 128], BF16, kind="ExternalInput").ap()
    Wf_d = nc.dram_tensor("Wf", [128, 160], BF16, kind="ExternalInput").ap()
    th1_d = nc.dram_tensor("th1", [128, 1], F32, kind="ExternalInput").ap()
    b2_d = nc.dram_tensor("bias2", [128, 1], F32, kind="ExternalInput").ap()
    th3_d = nc.dram_tensor("th3", [128, 1], F32, kind="ExternalInput").ap()
    b4_d = nc.dram_tensor("bias4", [128, 1], F32, kind="ExternalInput").ap()
    bf_d = nc.dram_tensor("bfv", [10, 1], F32, kind="ExternalInput").ap()
    id_d = nc.dram_tensor("ident", [128, 128], F32, kind="ExternalInput").ap()

    y_d = nc.dram_tensor("y", [B_CORE, 10], F32, kind="ExternalOutput").ap()

    with tile.TileContext(nc) as tc:
        with (
            tc.tile_pool(name="consts", bufs=1) as consts,
            tc.tile_pool(name="xin", bufs=3) as xin_pool,
            tc.tile_pool(name="xt", bufs=2) as xt_pool,
            tc.tile_pool(name="tl1", bufs=3) as tl1_pool,
            tc.tile_pool(name="s1", bufs=2) as s1_pool,
            tc.tile_pool(name="qq", bufs=2) as q_pool,
            tc.tile_pool(name="t3", bufs=3) as t3_pool,
            tc.tile_pool(name="s3", bufs=2) as s3_pool,
            tc.tile_pool(name="s4", bufs=2) as s4_pool,
            tc.tile_pool(name="oc", bufs=2) as oc_pool,
            tc.tile_pool(name="psA", bufs=3, space="PSUM") as psA_pool,   # [128,384]
            tc.tile_pool(name="psB", bufs=2, space="PSUM") as psB_pool,   # [128,512]
            tc.tile_pool(name="psC", bufs=3, space="PSUM") as psC_pool,   # [128,512]
        ):
            # --- load constants ---
            A1_s = consts.tile([128, 32 * 128], F32R)
            nc.sync.dma_start(out=A1_s, in_=A1_d)
            A1b_s = consts.tile([128, 32 * 128], BF16)
            nc.sync.dma_start(out=A1b_s, in_=A1b_d)
            W2_s = consts.tile([128, 128], BF16)
            nc.sync.dma_start(out=W2_s, in_=W2_d)
            W3a_s = consts.tile([128, 256], BF16)
            nc.sync.dma_start(out=W3a_s, in_=W3a_d)
            W3b_s = consts.tile([128, 256], BF16)
            nc.sync.dma_start(out=W3b_s, in_=W3b_d)
            W4_s = consts.tile([128, 6 * 128], BF16)
            nc.sync.dma_start(out=W4_s, in_=W4_d)
            Wf_s = consts.tile([128, 160], BF16)
            nc.sync.dma_start(out=Wf_s, in_=Wf_d)
            th1_s = consts.tile([128, 1], F32)
            nc.sync.dma_start(out=th1_s, in_=th1_d)
            b2_s = consts.tile([128, 1], F32)
            nc.sync.dma_start(out=b2_s, in_=b2_d)
            th3_s = consts.tile([128, 1], F32)
            nc.sync.dma_start(out=th3_s, in_=th3_d)
            b4_s = consts.tile([128, 1], F32)
            nc.sync.dma_start(out=b4_s, in_=b4_d)
            bf_s = consts.tile([10, 1], F32)
            nc.sync.dma_start(out=bf_s, in_=bf_d)
            id_s = consts.tile([128, 128], F32)
            nc.sync.dma_start(out=id_s, in_=id_d)

            for c in range(N_CHUNKS):
                xc = x_d[c * NB:(c + 1) * NB].rearrange("b h w -> (b h) w")

                # ---- transpose x chunk: [384 bh, 128 w] -> xT [128 w, 384] --
                ps_tr = psA_pool.tile([128, BH], F32, tag="psA")
                for t in range(3):
                    xnat = xin_pool.tile([128, 128], F32)
                    nc.sync.dma_start(out=xnat, in_=xc[128 * t:128 * (t + 1), :])
                    nc.tensor.transpose(ps_tr[:, 128 * t:128 * (t + 1)],
                                        xnat, id_s)
                xTh = xt_pool.tile([128, BH], BF16, tag="xTh")
                nc.vector.tensor_copy(xTh, ps_tr)
                xTl = xt_pool.tile([128, BH], F32R, tag="xTl")
                nc.vector.tensor_tensor(xTl, ps_tr, xTh, op=ALU.subtract)

                # ---- layer 1: 16 u-tiles x (even,odd) matmuls, pool, sign --
                s1t = s1_pool.tile([128, BH * 16], BF16)   # rows (p,ci), cols (bh,u)
                s1v = s1t.rearrange("p (bh u) -> p bh u", u=16)
                for m in range(16):
                    psa = psA_pool.tile([128, BH], F32, tag="psA")
                    psb = psA_pool.tile([128, BH], F32, tag="psA")
                    sa = slice((2 * m) * 128, (2 * m + 1) * 128)
                    sb = slice((2 * m + 1) * 128, (2 * m + 2) * 128)
                    nc.tensor.matmul(psa, A1b_s[:, sa], xTh, start=True, stop=False)
                    nc.tensor.matmul(psa, A1_s[:, sa], xTl, start=False, stop=True)
                    nc.tensor.matmul(psb, A1b_s[:, sb], xTh, start=True, stop=False)
                    nc.tensor.matmul(psb, A1_s[:, sb], xTl, start=False, stop=True)
                    sbb = tl1_pool.tile([128, BH], F32, tag="sbb")
                    nc.scalar.copy(sbb, psb)
                    tmp = tl1_pool.tile([128, BH], F32, tag="tmp")
                    nc.vector.tensor_tensor(tmp, psa, sbb, op=ALU.max)
                    nc.vector.tensor_scalar(out=s1v[:, :, m], in0=tmp,
                                            scalar1=th1_s, scalar2=0.5,
                                            op0=ALU.is_ge, op1=ALU.subtract)
                # zero the pad slots: (p=0, u=0) -> w=-1 ; (p=3, u=15) -> w=32
                nc.gpsimd.memset(s1v[0:32, :, 0], 0.0)
                nc.gpsimd.memset(s1v[96:128, :, 15], 0.0)

                # ---- layer 2: one K=128 matmul per 512-col sub-chunk -------
                qt = q_pool.tile([128, BH * 18], BF16)     # cols (bh, u') u'=u+1
                qv = qt.rearrange("p (bh u) -> p bh u", u=18)
                nc.gpsimd.memset(qv[:, :, 0], 0.0)
                nc.gpsimd.memset(qv[:, :, 17], 0.0)
                for s in range(12):
                    ps2 = psB_pool.tile([128, 512], F32, tag="psB")
                    nc.tensor.matmul(ps2, W2_s,
                                     s1t[:, 512 * s:512 * (s + 1)],
                                     start=True, stop=True)
                    nc.scalar.activation(
                        qv[:, 32 * s:32 * (s + 1), 1:17],
                        ps2.rearrange("p (a b) -> p a b", b=16),
                        AF.Sign, bias=b2_s)

                # ---- layer 3: 2 M-tiles x (K=128 + K=64) per sub-chunk -----
                s3t = s3_pool.tile([128, BH * 16], BF16)   # [co3, (bh,u)]
                for s in range(12):
                    p3a = psC_pool.tile([128, 512], F32, tag="psC")
                    p3b = psC_pool.tile([128, 512], F32, tag="psC")
                    q_mid = qv[:, 32 * s:32 * (s + 1), 1:17]
                    nc.tensor.matmul(p3a, W3a_s[:, 0:128], q_mid, start=True,
                                     stop=False)
                    nc.tensor.matmul(p3a, W3b_s[64:128, 0:128],
                                     qv[64:128, 32 * s:32 * (s + 1), 0:16],
                                     start=False, stop=True)
                    nc.tensor.matmul(p3b, W3a_s[:, 128:256], q_mid, start=True,
                                     stop=False)
                    nc.tensor.matmul(p3b, W3b_s[0:64, 128:256],
                                     qv[0:64, 32 * s:32 * (s + 1), 2:18],
                                     start=False, stop=True)
                    sb3b = t3_pool.tile([128, 512], BF16, tag="sb3b")
                    nc.scalar.copy(sb3b, p3b)
                    tmp3 = t3_pool.tile([128, 512], BF16, tag="tmp3")
                    nc.vector.tensor_tensor(tmp3, p3a, sb3b, op=ALU.max)
                    nc.vector.tensor_scalar(out=s3t[:, 512 * s:512 * (s + 1)],
                                            in0=tmp3, scalar1=th3_s, scalar2=0.5,
                                            op0=ALU.is_ge, op1=ALU.subtract)

                # ---- layer 4: contract (ci, h); 2 N-halves of 512 ----------
                s4t = s4_pool.tile([128, NB * 16], BF16)   # [co4, (b,w)]
                s3v = s3t.rearrange("p (b h u) -> p b h u", h=6, u=16)
                for half in range(2):
                    bsl = slice(32 * half, 32 * (half + 1))
                    ps4 = psB_pool.tile([128, 512], F32, tag="psB")
                    for h in range(6):
                        nc.tensor.matmul(
                            ps4, W4_s[:, 128 * h:128 * (h + 1)],
                            s3v[:, bsl, h, :],
                            start=(h == 0), stop=(h == 5))
                    nc.scalar.activation(s4t[:, 512 * half:512 * (half + 1)],
                                         ps4, AF.Sign, bias=b4_s)

                # ---- fc ----------------------------------------------------
                s4v = s4t.rearrange("p (b w) -> p b w", w=16)
                psf = psA_pool.tile([10, 64], F32, tag="psA")
                for w in range(16):
                    nc.tensor.matmul(psf, Wf_s[:, 10 * w:10 * (w + 1)],
                                     s4v[:, :, w:w + 1], start=(w == 0), stop=(w == 15))
                outc = oc_pool.tile([10, NB], F32)
                nc.vector.tensor_scalar_add(outc, psf, bf_s)
                nc.sync.dma_start(
                    out=y_d[c * NB:(c + 1) * NB, :].rearrange("b j -> j b"),
                    in_=outc)

    nc.compile()
    return nc


_PROGRAM = None


def _get_program():
    global _PROGRAM
    if _PROGRAM is None:
        _PROGRAM = build_program()
    return _PROGRAM


def run(trace=False, **inputs):
    inputs = {k: np.asarray(v) for k, v in inputs.items()}
    consts = prepare_host_tensors(
        **{k: inputs[k] for k in
           ("w1", "b1", "w2", "b2", "w3", "b3", "w4", "b4",
            "g1", "be1", "m1", "v1", "g2", "be2", "m2", "v2",
            "g3", "be3", "m3", "v3", "g4", "be4", "m4", "v4", "wf", "bf")})
    x = inputs["x"].astype(np.float32)           # [8192, 1, 6, 128]
    nc = _get_program()
    in_maps = []
    for k in range(N_CORES):
        m = {"x": np.ascontiguousarray(x[k * B_CORE:(k + 1) * B_CORE, 0])}
        m.update(consts)
        in_maps.append(m)
    res = run_bass_kernel_spmd(nc, in_maps, list(range(N_CORES)), trace=trace)
    y = np.concatenate([r["y"] for r in res.results], axis=0)
    return y.astype(np.float32), res


def kernel(**inputs):
    y, _ = run(trace=False, **inputs)
    return y



# revision 6
# speedup vs baseline: 1.3659x; 1.3659x over previous
"""Trainium2 Bass kernel for the binarized CNN (nn_CNN_binary_55001351193058).

Pure data-parallel over 8 NeuronCores (batch-sharded, 1024 samples/core).

Pipeline (per 64-sample chunk, all layouts (u, b, h)-column-major):
  - x is transposed + split hi/lo bf16 on the host; DMA'd as [128 w, cols].
  - L1: 8 u-tiles (4 pooled positions x 32ch rows), even/odd conv outputs as
    separate PSUM tiles, hi+lo bf16 matmuls sharing one stationary.
    Eviction: DVE tensor_scalar (>= th, -0.5) per half, GpSimd max -> s1 (+-0.5).
  - L2: shift-invariant even/odd stationaries + K=32 boundary matmuls;
    DVE tensor_scalar -> q (+-0.5), zero-padded u'=0/17 halo.
  - L3: 16 position-pair tiles; even u evicted via 2x ACT Sign (+-1),
    odd u via 2x DVE tensor_scalar (+-0.5); DVE max -> s3.
  - L4: contract (ci,h) split by u-parity (separate biases absorb the
    +-1 / +-0.5 scale), ACT Sign -> s4 (+-1).
  - fc: 16 accumulating matmuls, DVE bias add, DMA out.
Exact small-integer arithmetic in layers 2-4 + fc; BatchNorm+Hardtanh+binarize
folded into per-channel thresholds computed on the host in float64.
"""

import numpy as np
import ml_dtypes

import concourse.bass as bass
import concourse.mybir as mybir
import concourse.tile as tile
from concourse import bacc
from concourse.bass_utils import run_bass_kernel_spmd

F32 = mybir.dt.float32
F16 = mybir.dt.float16
BF16 = mybir.dt.bfloat16
AF = mybir.ActivationFunctionType
ALU = mybir.AluOpType

B_TOTAL = 8192
N_CORES = 8
B_CORE = B_TOTAL // N_CORES          # 1024
NB = 64                              # samples per chunk
N_CHUNKS = B_CORE // NB              # 16
BH = NB * 6                          # 384 (b,h) columns per chunk
EPS = 1e-5

bf16 = ml_dtypes.bfloat16


# ----------------------------------------------------------------------------
# Host-side weight preparation (float64 where it matters)
# ----------------------------------------------------------------------------

def _sgn(w):
    return np.where(w >= 0, 1.0, -1.0)


def _threshold(g, be, m, v, bias):
    inv = g.astype(np.float64) / np.sqrt(v.astype(np.float64) + EPS)
    assert (inv > 0).all(), "BN scale must be positive for threshold folding"
    sh = be.astype(np.float64) - m.astype(np.float64) * inv
    return -bias.astype(np.float64) - sh / inv


def _check_margin(th, grid_step, name):
    d = np.abs(th / grid_step - np.round(th / grid_step)) * grid_step
    if d.min() < 1e-4:
        raise AssertionError(f"threshold margin too small for {name}: {d.min()}")


def prepare_host_tensors(w1, b1, w2, b2, w3, b3, w4, b4,
                         g1, be1, m1, v1, g2, be2, m2, v2,
                         g3, be3, m3, v3, g4, be4, m4, v4, wf, bf):
    t1 = _threshold(g1, be1, m1, v1, b1)       # [32]
    t2 = _threshold(g2, be2, m2, v2, b2)       # [64]
    t3 = _threshold(g3, be3, m3, v3, b3)       # [128]
    t4 = _threshold(g4, be4, m4, v4, b4)       # [128]
    _check_margin(t2 / 2.0, 0.5, "t2")         # z2 ints when s1 = +-0.5
    _check_margin(t3, 2.0, "t3")               # z3 ints (q = +-0.5)
    _check_margin(t4 / 2.0, 1.0, "t4")

    s1 = _sgn(w1)[:, 0, 0, :].astype(np.float32)        # [32, 9]
    s2 = _sgn(w2)[:, :, 0, :].astype(np.float32)        # [64, 32, 3]
    s3 = _sgn(w3)[:, :, 0, :].astype(np.float32)        # [128, 64, 3]
    s4 = _sgn(w4)[:, :, :, 0].astype(np.float32)        # [128, 128, 6]
    sf = _sgn(wf).astype(np.float32)                    # [10, 2048]

    # L1: 16 m-tiles (8 u x even/odd), lhsT [w, (p,ci)].
    # row (p,ci) of tile (u,half) holds conv1 out at wy = 2*(4u+p)+half:
    #   wx = 2*wy + k - 4
    A1 = np.zeros((16, 128, 128), np.float32)
    for u in range(8):
        for half in range(2):
            m = 2 * u + half
            for p in range(4):
                wy = 2 * (4 * u + p) + half
                for k in range(9):
                    wx = 2 * wy + k - 4
                    if 0 <= wx < 128:
                        A1[m, wx, p * 32:(p + 1) * 32] = s1[:, k]
    A1 = A1.transpose(1, 0, 2).reshape(128, 16 * 128)

    # L2 stationaries, lhsT [(p,ci), (op,co)].
    # even v (out pos 4u+op):  k = p - op + 1
    # odd  v (out pos 4u+2+op): k = p - op - 1
    W2e = np.zeros((128, 128), np.float32)
    W2o = np.zeros((128, 128), np.float32)
    for p in range(4):
        for op in range(2):
            ke = p - op + 1
            if 0 <= ke <= 2:
                W2e[p * 32:(p + 1) * 32, op * 64:(op + 1) * 64] = s2[:, :, ke].T
            ko = p - op - 1
            if 0 <= ko <= 2:
                W2o[p * 32:(p + 1) * 32, op * 64:(op + 1) * 64] = s2[:, :, ko].T
    # boundary taps: even v op0 k0 from prev tile p3; odd v op1 k2 from next p0
    W2eb = np.zeros((128, 128), np.float32)
    W2eb[96:128, 0:64] = s2[:, :, 0].T
    W2ob = np.zeros((128, 128), np.float32)
    W2ob[0:32, 64:128] = s2[:, :, 2].T

    # L3 stationaries (as in the 2-op3-tile scheme), q rows (op, co2).
    W3a = np.zeros((128, 256), np.float32)
    W3a[0:64, 0:128] = s3[:, :, 1].T
    W3a[64:128, 0:128] = s3[:, :, 2].T
    W3a[0:64, 128:256] = s3[:, :, 0].T
    W3a[64:128, 128:256] = s3[:, :, 1].T
    W3b = np.zeros((128, 256), np.float32)
    W3b[64:128, 0:128] = s3[:, :, 0].T     # even tile bnd: odd[u-1], k=0
    W3b[0:64, 128:256] = s3[:, :, 2].T     # odd tile bnd: even[u+1], k=2

    W4t = s4.transpose(2, 1, 0).reshape(6, 128, 128)    # [h][ci, co]
    W4t = W4t.transpose(1, 0, 2).reshape(128, 6 * 128)  # [ci, (h,co)]

    Wf = sf.reshape(10, 128, 16)                         # [j, co, w]
    Wf = Wf.transpose(1, 2, 0).reshape(128, 160)         # [co, (w,j)]

    th1 = np.tile(t1, 4).astype(np.float32).reshape(128, 1)          # (p,ci)
    nb1 = (-th1).astype(np.float32)                                  # ACT bias
    th2v = np.concatenate([t2, t2]).astype(np.float32).reshape(128, 1) / 2.0
    th3v = (t3 / 2.0).astype(np.float32).reshape(128, 1)             # DVE odd u
    b3v = (-t3 / 2.0).astype(np.float32).reshape(128, 1)             # ACT even u
    b4e = (-t4).astype(np.float32).reshape(128, 1)                   # s3 = +-1
    b4o = (-t4 / 2.0).astype(np.float32).reshape(128, 1)             # s3 = +-0.5
    bfv = bf.astype(np.float32).reshape(10, 1)

    return dict(
        A1=A1.astype(np.float16),
        W2e=W2e.astype(bf16), W2o=W2o.astype(bf16),
        W2eb=W2eb.astype(bf16), W2ob=W2ob.astype(bf16),
        W3a=W3a.astype(bf16), W3b=W3b.astype(bf16),
        W4t=W4t.astype(bf16), Wf=Wf.astype(bf16),
        th1=th1, nb1=nb1, th2v=th2v, th3v=th3v, b3v=b3v, b4e=b4e, b4o=b4o,
        bfv=bfv,
    )


# ----------------------------------------------------------------------------
# Bass program (identical SPMD program for each core)
# ----------------------------------------------------------------------------

def build_program():
    nc = bacc.Bacc("TRN2", target_bir_lowering=False, debug=False)

    xh_d = nc.dram_tensor("xh", [128, B_CORE * 6], F16, kind="ExternalInput").ap()
    xl_d = nc.dram_tensor("xl", [128, B_CORE * 6], F16, kind="ExternalInput").ap()
    A1_d = nc.dram_tensor("A1", [128, 16 * 128], F16, kind="ExternalInput").ap()
    W2e_d = nc.dram_tensor("W2e", [128, 128], BF16, kind="ExternalInput").ap()
    W2o_d = nc.dram_tensor("W2o", [128, 128], BF16, kind="ExternalInput").ap()
    W2eb_d = nc.dram_tensor("W2eb", [128, 128], BF16, kind="ExternalInput").ap()
    W2ob_d = nc.dram_tensor("W2ob", [128, 128], BF16, kind="ExternalInput").ap()
    W3a_d = nc.dram_tensor("W3a", [128, 256], BF16, kind="ExternalInput").ap()
    W3b_d = nc.dram_tensor("W3b", [128, 256], BF16, kind="ExternalInput").ap()
    W4_d = nc.dram_tensor("W4t", [128, 6 * 128], BF16, kind="ExternalInput").ap()
    Wf_d = nc.dram_tensor("Wf", [128, 160], BF16, kind="ExternalInput").ap()
    th1_d = nc.dram_tensor("th1", [128, 1], F32, kind="ExternalInput").ap()
    nb1_d = nc.dram_tensor("nb1", [128, 1], F32, kind="ExternalInput").ap()
    th2_d = nc.dram_tensor("th2v", [128, 1], F32, kind="ExternalInput").ap()
    th3_d = nc.dram_tensor("th3v", [128, 1], F32, kind="ExternalInput").ap()
    b3_d = nc.dram_tensor("b3v", [128, 1], F32, kind="ExternalInput").ap()
    b4e_d = nc.dram_tensor("b4e", [128, 1], F32, kind="ExternalInput").ap()
    b4o_d = nc.dram_tensor("b4o", [128, 1], F32, kind="ExternalInput").ap()
    bf_d = nc.dram_tensor("bfv", [10, 1], F32, kind="ExternalInput").ap()

    y_d = nc.dram_tensor("y", [B_CORE, 10], F32, kind="ExternalOutput").ap()

    with tile.TileContext(nc) as tc:
        with (
            tc.tile_pool(name="consts", bufs=1) as consts,
            tc.tile_pool(name="xin", bufs=4) as xin_pool,
            tc.tile_pool(name="greadyE", bufs=3) as gE_pool,
            tc.tile_pool(name="greadyO", bufs=3) as gO_pool,
            tc.tile_pool(name="s1buf", bufs=2) as s1_pool,
            tc.tile_pool(name="qbuf", bufs=2) as q_pool,
            tc.tile_pool(name="fa", bufs=3) as fa_pool,
            tc.tile_pool(name="fb", bufs=3) as fb_pool,
            tc.tile_pool(name="s3buf", bufs=2) as s3_pool,
            tc.tile_pool(name="s4buf", bufs=2) as s4_pool,
            tc.tile_pool(name="oc", bufs=2) as oc_pool,
            tc.tile_pool(name="psMain", bufs=6, space="PSUM") as psM_pool,
            tc.tile_pool(name="psL4", bufs=2, space="PSUM") as psL4_pool,
        ):
            # --- constants ---
            A1_s = consts.tile([128, 16 * 128], F16)
            nc.sync.dma_start(out=A1_s, in_=A1_d)
            W2e_s = consts.tile([128, 128], BF16)
            nc.sync.dma_start(out=W2e_s, in_=W2e_d)
            W2o_s = consts.tile([128, 128], BF16)
            nc.sync.dma_start(out=W2o_s, in_=W2o_d)
            W2eb_s = consts.tile([128, 128], BF16)
            nc.sync.dma_start(out=W2eb_s, in_=W2eb_d)
            W2ob_s = consts.tile([128, 128], BF16)
            nc.sync.dma_start(out=W2ob_s, in_=W2ob_d)
            W3a_s = consts.tile([128, 256], BF16)
            nc.sync.dma_start(out=W3a_s, in_=W3a_d)
            W3b_s = consts.tile([128, 256], BF16)
            nc.sync.dma_start(out=W3b_s, in_=W3b_d)
            W4_s = consts.tile([128, 6 * 128], BF16)
            nc.sync.dma_start(out=W4_s, in_=W4_d)
            Wf_s = consts.tile([128, 160], BF16)
            nc.sync.dma_start(out=Wf_s, in_=Wf_d)
            th1_s = consts.tile([128, 1], F32)
            nc.sync.dma_start(out=th1_s, in_=th1_d)
            nb1_s = consts.tile([128, 1], F32)
            nc.sync.dma_start(out=nb1_s, in_=nb1_d)
            th2_s = consts.tile([128, 1], F32)
            nc.sync.dma_start(out=th2_s, in_=th2_d)
            th3_s = consts.tile([128, 1], F32)
            nc.sync.dma_start(out=th3_s, in_=th3_d)
            b3_s = consts.tile([128, 1], F32)
            nc.sync.dma_start(out=b3_s, in_=b3_d)
            b4e_s = consts.tile([128, 1], F32)
            nc.sync.dma_start(out=b4e_s, in_=b4e_d)
            b4o_s = consts.tile([128, 1], F32)
            nc.sync.dma_start(out=b4o_s, in_=b4o_d)
            bf_s = consts.tile([10, 1], F32)
            nc.sync.dma_start(out=bf_s, in_=bf_d)

            # persistent double buffers (pads zeroed once)
            s1_bufs = []
            q_bufs = []
            s3_bufs = []
            s4_bufs = []
            for i in range(2):
                s1b = s1_pool.tile([128, 8 * BH], BF16, name=f"s1b{i}")
                s1_bufs.append(s1b)
                qb = q_pool.tile([128, 18 * BH], BF16, name=f"qb{i}")
                nc.vector.memset(qb[:, 0:BH], 0.0)
                nc.vector.memset(qb[:, 17 * BH:18 * BH], 0.0)
                q_bufs.append(qb)
                s3b = s3_pool.tile([128, 16 * BH], BF16, name=f"s3b{i}")
                s3_bufs.append(s3b)
                s4b = s4_pool.tile([128, 1024], BF16, name=f"s4b{i}")
                s4_bufs.append(s4b)

            for c in range(N_CHUNKS):
                s1t = s1_bufs[c % 2]
                qt = q_bufs[c % 2]
                s3t = s3_bufs[c % 2]
                s4t = s4_bufs[c % 2]
                csl = slice(c * BH, (c + 1) * BH)

                xh_t = xin_pool.tile([128, BH], F16, tag="xh")
                nc.sync.dma_start(out=xh_t, in_=xh_d[:, csl])
                xl_t = xin_pool.tile([128, BH], F16, tag="xl")
                nc.sync.dma_start(out=xl_t, in_=xl_d[:, csl])

                # ---- L1: 8 u-tiles, even/odd pairs, hi+lo accumulation ----
                for u in range(8):
                    psE = psM_pool.tile([128, BH], F32, tag="psM")
                    psO = psM_pool.tile([128, BH], F32, tag="psM")
                    se = slice((2 * u) * 128, (2 * u + 1) * 128)
                    so = slice((2 * u + 1) * 128, (2 * u + 2) * 128)
                    nc.tensor.matmul(psE, A1_s[:, se], xh_t, start=True, stop=False)
                    nc.tensor.matmul(psE, A1_s[:, se], xl_t, start=False, stop=True)
                    nc.tensor.matmul(psO, A1_s[:, so], xh_t, start=True, stop=False)
                    nc.tensor.matmul(psO, A1_s[:, so], xl_t, start=False, stop=True)
                    gE = gE_pool.tile([128, BH], BF16, tag="gE")
                    nc.scalar.activation(gE, psE, AF.Sign, bias=nb1_s)
                    gO = gO_pool.tile([128, BH], BF16, tag="gO")
                    nc.vector.tensor_scalar(out=gO, in0=psO, scalar1=th1_s,
                                            scalar2=0.5, op0=ALU.is_ge,
                                            op1=ALU.subtract)
                    nc.vector.scalar_tensor_tensor(
                        s1t[:, u * BH:(u + 1) * BH], gE, 0.5, gO,
                        op0=ALU.mult, op1=ALU.max)

                # ---- L2: 16 v-tiles (out positions 2v, 2v+1) --------------
                for v in range(16):
                    u = v // 2
                    usl = slice(u * BH, (u + 1) * BH)
                    ps2 = psM_pool.tile([128, BH], F32, tag="psM")
                    if v % 2 == 0:
                        nc.tensor.matmul(ps2, W2e_s, s1t[:, usl],
                                         start=True, stop=(u == 0))
                        if u > 0:
                            nc.tensor.matmul(
                                ps2, W2eb_s[96:128, :],
                                s1t[96:128, (u - 1) * BH:u * BH],
                                start=False, stop=True,
                                tile_position=(96, 0))
                    else:
                        nc.tensor.matmul(ps2, W2o_s, s1t[:, usl],
                                         start=True, stop=(u == 7))
                        if u < 7:
                            nc.tensor.matmul(
                                ps2, W2ob_s[0:32, :],
                                s1t[0:32, (u + 1) * BH:(u + 2) * BH],
                                start=False, stop=True)
                    nc.vector.tensor_scalar(
                        out=qt[:, (v + 1) * BH:(v + 2) * BH], in0=ps2,
                        scalar1=th2_s, scalar2=0.5,
                        op0=ALU.is_ge, op1=ALU.subtract)

                # ---- L3: 16 pooled-position tiles -------------------------
                for u in range(16):
                    mid = slice((u + 1) * BH, (u + 2) * BH)
                    p3a = psM_pool.tile([128, BH], F32, tag="psM")
                    p3b = psM_pool.tile([128, BH], F32, tag="psM")
                    nc.tensor.matmul(p3a, W3a_s[:, 0:128], qt[:, mid],
                                     start=True, stop=False)
                    nc.tensor.matmul(p3a, W3b_s[64:128, 0:128],
                                     qt[64:128, u * BH:(u + 1) * BH],
                                     start=False, stop=True)
                    nc.tensor.matmul(p3b, W3a_s[:, 128:256], qt[:, mid],
                                     start=True, stop=False)
                    nc.tensor.matmul(p3b, W3b_s[0:64, 128:256],
                                     qt[0:64, (u + 2) * BH:(u + 3) * BH],
                                     start=False, stop=True)
                    fa = fa_pool.tile([128, BH], BF16, tag="fa")
                    nc.scalar.activation(fa, p3a, AF.Sign, bias=b3_s)
                    fb = fb_pool.tile([128, BH], BF16, tag="fb")
                    nc.vector.tensor_scalar(out=fb, in0=p3b, scalar1=th3_s,
                                            scalar2=0.5, op0=ALU.is_ge,
                                            op1=ALU.subtract)
                    nc.vector.scalar_tensor_tensor(
                        s3t[:, u * BH:(u + 1) * BH], fa, 0.5, fb,
                        op0=ALU.mult, op1=ALU.max)

                # ---- L4: contract (ci,h), split by u-parity ---------------
                s3v = s3t.rearrange("p (u b h) -> p u b h", b=NB, h=6)
                for half in range(2):
                    ps4 = psL4_pool.tile([128, 512], F32, tag="ps4")
                    for h in range(6):
                        nc.tensor.matmul(
                            ps4, W4_s[:, 128 * h:128 * (h + 1)],
                            s3v[:, 8 * half:8 * (half + 1), :, h],
                            start=(h == 0), stop=(h == 5))
                    nc.scalar.activation(s4t[:, 512 * half:512 * (half + 1)],
                                         ps4, AF.Sign, bias=b4o_s)

                # ---- fc ---------------------------------------------------
                psf = psL4_pool.tile([10, 64], F32, tag="ps4")
                for w in range(16):
                    blk = w * 64
                    nc.tensor.matmul(psf, Wf_s[:, 10 * w:10 * (w + 1)],
                                     s4t[:, blk:blk + 64],
                                     start=(w == 0), stop=(w == 15))
                outc = oc_pool.tile([10, NB], F32)
                nc.vector.tensor_scalar_add(outc, psf, bf_s)
                nc.sync.dma_start(
                    out=y_d[c * NB:(c + 1) * NB, :].rearrange("b j -> j b"),
                    in_=outc)

    nc.compile()
    return nc


_PROGRAM = None


def _get_program():
    global _PROGRAM
    if _PROGRAM is None:
        _PROGRAM = build_program()
    return _PROGRAM


def run(trace=False, **inputs):
    inputs = {k: np.asarray(v) for k, v in inputs.items()}
    consts = prepare_host_tensors(
        **{k: inputs[k] for k in
           ("w1", "b1", "w2", "b2", "w3", "b3", "w4", "b4",
            "g1", "be1", "m1", "v1", "g2", "be2", "m2", "v2",
            "g3", "be3", "m3", "v3", "g4", "be4", "m4", "v4", "wf", "bf")})
    x = inputs["x"].astype(np.float32)           # [8192, 1, 6, 128]
    nc = _get_program()
    in_maps = []
    for k in range(N_CORES):
        xc = x[k * B_CORE:(k + 1) * B_CORE, 0]               # [1024, 6, 128]
        xT = np.ascontiguousarray(xc.reshape(B_CORE * 6, 128).T)  # [128, 6144]
        xh = xT.astype(np.float16)
        xl = (xT - xh.astype(np.float32)).astype(np.float16)
        m = {"xh": xh, "xl": xl}
        m.update(consts)
        in_maps.append(m)
    res = run_bass_kernel_spmd(nc, in_maps, list(range(N_CORES)), trace=trace)
    y = np.concatenate([r["y"] for r in res.results], axis=0)
    return y.astype(np.float32), res


def kernel(**inputs):
    y, _ = run(trace=False, **inputs)
    return y


# revision 7
# speedup vs baseline: 1.4705x; 1.0766x over previous
"""Trainium2 Bass kernel for the binarized CNN (nn_CNN_binary_55001351193058).

Pure data-parallel over 8 NeuronCores (batch-sharded, 1024 samples/core).

Pipeline (per 64-sample chunk, all layouts (u, b, h)-column-major):
  - x is transposed + split hi/lo bf16 on the host; DMA'd as [128 w, cols].
  - L1: 8 u-tiles (4 pooled positions x 32ch rows), even/odd conv outputs as
    separate PSUM tiles, hi+lo bf16 matmuls sharing one stationary.
    Eviction: DVE tensor_scalar (>= th, -0.5) per half, GpSimd max -> s1 (+-0.5).
  - L2: shift-invariant even/odd stationaries + K=32 boundary matmuls;
    DVE tensor_scalar -> q (+-0.5), zero-padded u'=0/17 halo.
  - L3: 16 position-pair tiles; even u evicted via 2x ACT Sign (+-1),
    odd u via 2x DVE tensor_scalar (+-0.5); DVE max -> s3.
  - L4: contract (ci,h) split by u-parity (separate biases absorb the
    +-1 / +-0.5 scale), ACT Sign -> s4 (+-1).
  - fc: 16 accumulating matmuls, DVE bias add, DMA out.
Exact small-integer arithmetic in layers 2-4 + fc; BatchNorm+Hardtanh+binarize
folded into per-channel thresholds computed on the host in float64.
"""

import numpy as np
import ml_dtypes

import concourse.bass as bass
import concourse.mybir as mybir
import concourse.tile as tile
from concourse import bacc
from concourse.bass_utils import run_bass_kernel_spmd

F32 = mybir.dt.float32
F16 = mybir.dt.float16
BF16 = mybir.dt.bfloat16
AF = mybir.ActivationFunctionType
ALU = mybir.AluOpType

B_TOTAL = 8192
N_CORES = 8
B_CORE = B_TOTAL // N_CORES          # 1024
NB = 64                              # samples per chunk
N_CHUNKS = B_CORE // NB              # 16
BH = NB * 6                          # 384 (b,h) columns per chunk
EPS = 1e-5

bf16 = ml_dtypes.bfloat16


# ----------------------------------------------------------------------------
# Host-side weight preparation (float64 where it matters)
# ----------------------------------------------------------------------------

def _sgn(w):
    return np.where(w >= 0, 1.0, -1.0)


def _threshold(g, be, m, v, bias):
    inv = g.astype(np.float64) / np.sqrt(v.astype(np.float64) + EPS)
    assert (inv > 0).all(), "BN scale must be positive for threshold folding"
    sh = be.astype(np.float64) - m.astype(np.float64) * inv
    return -bias.astype(np.float64) - sh / inv


def _check_margin(th, grid_step, name):
    d = np.abs(th / grid_step - np.round(th / grid_step)) * grid_step
    if d.min() < 1e-4:
        raise AssertionError(f"threshold margin too small for {name}: {d.min()}")


def prepare_host_tensors(w1, b1, w2, b2, w3, b3, w4, b4,
                         g1, be1, m1, v1, g2, be2, m2, v2,
                         g3, be3, m3, v3, g4, be4, m4, v4, wf, bf):
    t1 = _threshold(g1, be1, m1, v1, b1)       # [32]
    t2 = _threshold(g2, be2, m2, v2, b2)       # [64]
    t3 = _threshold(g3, be3, m3, v3, b3)       # [128]
    t4 = _threshold(g4, be4, m4, v4, b4)       # [128]
    _check_margin(t2 / 2.0, 0.5, "t2")         # z2 ints when s1 = +-0.5
    _check_margin(t3, 2.0, "t3")               # z3 ints (q = +-0.5)
    _check_margin(t4 / 2.0, 1.0, "t4")

    s1 = _sgn(w1)[:, 0, 0, :].astype(np.float32)        # [32, 9]
    s2 = _sgn(w2)[:, :, 0, :].astype(np.float32)        # [64, 32, 3]
    s3 = _sgn(w3)[:, :, 0, :].astype(np.float32)        # [128, 64, 3]
    s4 = _sgn(w4)[:, :, :, 0].astype(np.float32)        # [128, 128, 6]
    sf = _sgn(wf).astype(np.float32)                    # [10, 2048]

    # L1: 16 m-tiles (8 u x even/odd), lhsT [w, (p,ci)].
    # row (p,ci) of tile (u,half) holds conv1 out at wy = 2*(4u+p)+half:
    #   wx = 2*wy + k - 4
    A1 = np.zeros((16, 128, 128), np.float32)
    for u in range(8):
        for half in range(2):
            m = 2 * u + half
            for p in range(4):
                wy = 2 * (4 * u + p) + half
                for k in range(9):
                    wx = 2 * wy + k - 4
                    if 0 <= wx < 128:
                        A1[m, wx, p * 32:(p + 1) * 32] = s1[:, k]
    A1 = A1.transpose(1, 0, 2).reshape(128, 16 * 128)

    # L2 stationaries, lhsT [(p,ci), (op,co)].
    # even v (out pos 4u+op):  k = p - op + 1
    # odd  v (out pos 4u+2+op): k = p - op - 1
    W2e = np.zeros((128, 128), np.float32)
    W2o = np.zeros((128, 128), np.float32)
    for p in range(4):
        for op in range(2):
            ke = p - op + 1
            if 0 <= ke <= 2:
                W2e[p * 32:(p + 1) * 32, op * 64:(op + 1) * 64] = s2[:, :, ke].T
            ko = p - op - 1
            if 0 <= ko <= 2:
                W2o[p * 32:(p + 1) * 32, op * 64:(op + 1) * 64] = s2[:, :, ko].T
    # boundary taps: even v op0 k0 from prev tile p3; odd v op1 k2 from next p0
    W2eb = np.zeros((128, 128), np.float32)
    W2eb[96:128, 0:64] = s2[:, :, 0].T
    W2ob = np.zeros((128, 128), np.float32)
    W2ob[0:32, 64:128] = s2[:, :, 2].T

    # L3 stationaries (as in the 2-op3-tile scheme), q rows (op, co2).
    W3a = np.zeros((128, 256), np.float32)
    W3a[0:64, 0:128] = s3[:, :, 1].T
    W3a[64:128, 0:128] = s3[:, :, 2].T
    W3a[0:64, 128:256] = s3[:, :, 0].T
    W3a[64:128, 128:256] = s3[:, :, 1].T
    W3b = np.zeros((128, 256), np.float32)
    W3b[64:128, 0:128] = s3[:, :, 0].T     # even tile bnd: odd[u-1], k=0
    W3b[0:64, 128:256] = s3[:, :, 2].T     # odd tile bnd: even[u+1], k=2

    W4t = s4.transpose(2, 1, 0).reshape(6, 128, 128)    # [h][ci, co]
    W4t = W4t.transpose(1, 0, 2).reshape(128, 6 * 128)  # [ci, (h,co)]

    Wf = sf.reshape(10, 128, 16)                         # [j, co, w]
    Wf = Wf.transpose(1, 2, 0).reshape(128, 160)         # [co, (w,j)]

    th1 = np.tile(t1, 4).astype(np.float32).reshape(128, 1)          # (p,ci)
    nb1 = (-th1).astype(np.float32)                                  # ACT bias
    th2v = np.concatenate([t2, t2]).astype(np.float32).reshape(128, 1) / 2.0
    th3v = (t3 / 2.0).astype(np.float32).reshape(128, 1)             # DVE odd u
    b3v = (-t3 / 2.0).astype(np.float32).reshape(128, 1)             # ACT even u
    b4e = (-t4).astype(np.float32).reshape(128, 1)                   # s3 = +-1
    b4o = (-t4 / 2.0).astype(np.float32).reshape(128, 1)             # s3 = +-0.5
    bfv = bf.astype(np.float32).reshape(10, 1)

    return dict(
        A1=A1.astype(np.float16),
        W2e=W2e.astype(bf16), W2o=W2o.astype(bf16),
        W2eb=W2eb.astype(bf16), W2ob=W2ob.astype(bf16),
        W3a=W3a.astype(bf16), W3b=W3b.astype(bf16),
        W4t=W4t.astype(bf16), Wf=Wf.astype(bf16),
        th1=th1, nb1=nb1, th2v=th2v, th3v=th3v, b3v=b3v, b4e=b4e, b4o=b4o,
        bfv=bfv,
    )


# ----------------------------------------------------------------------------
# Bass program (identical SPMD program for each core)
# ----------------------------------------------------------------------------

def build_program():
    nc = bacc.Bacc("TRN2", target_bir_lowering=False, debug=False)

    xh_d = nc.dram_tensor("xh", [128, B_CORE * 6], F16, kind="ExternalInput").ap()
    xl_d = nc.dram_tensor("xl", [128, B_CORE * 6], F16, kind="ExternalInput").ap()
    A1_d = nc.dram_tensor("A1", [128, 16 * 128], F16, kind="ExternalInput").ap()
    W2e_d = nc.dram_tensor("W2e", [128, 128], BF16, kind="ExternalInput").ap()
    W2o_d = nc.dram_tensor("W2o", [128, 128], BF16, kind="ExternalInput").ap()
    W2eb_d = nc.dram_tensor("W2eb", [128, 128], BF16, kind="ExternalInput").ap()
    W2ob_d = nc.dram_tensor("W2ob", [128, 128], BF16, kind="ExternalInput").ap()
    W3a_d = nc.dram_tensor("W3a", [128, 256], BF16, kind="ExternalInput").ap()
    W3b_d = nc.dram_tensor("W3b", [128, 256], BF16, kind="ExternalInput").ap()
    W4_d = nc.dram_tensor("W4t", [128, 6 * 128], BF16, kind="ExternalInput").ap()
    Wf_d = nc.dram_tensor("Wf", [128, 160], BF16, kind="ExternalInput").ap()
    th1_d = nc.dram_tensor("th1", [128, 1], F32, kind="ExternalInput").ap()
    nb1_d = nc.dram_tensor("nb1", [128, 1], F32, kind="ExternalInput").ap()
    th2_d = nc.dram_tensor("th2v", [128, 1], F32, kind="ExternalInput").ap()
    th3_d = nc.dram_tensor("th3v", [128, 1], F32, kind="ExternalInput").ap()
    b3_d = nc.dram_tensor("b3v", [128, 1], F32, kind="ExternalInput").ap()
    b4e_d = nc.dram_tensor("b4e", [128, 1], F32, kind="ExternalInput").ap()
    b4o_d = nc.dram_tensor("b4o", [128, 1], F32, kind="ExternalInput").ap()
    bf_d = nc.dram_tensor("bfv", [10, 1], F32, kind="ExternalInput").ap()

    y_d = nc.dram_tensor("y", [B_CORE, 10], F32, kind="ExternalOutput").ap()

    with tile.TileContext(nc) as tc:
        with (
            tc.tile_pool(name="consts", bufs=1) as consts,
            tc.tile_pool(name="xin", bufs=4) as xin_pool,
            tc.tile_pool(name="greadyE", bufs=3) as gE_pool,
            tc.tile_pool(name="greadyO", bufs=3) as gO_pool,
            tc.tile_pool(name="s1buf", bufs=2) as s1_pool,
            tc.tile_pool(name="qbuf", bufs=2) as q_pool,
            tc.tile_pool(name="fa", bufs=3) as fa_pool,
            tc.tile_pool(name="fb", bufs=3) as fb_pool,
            tc.tile_pool(name="s3buf", bufs=2) as s3_pool,
            tc.tile_pool(name="s4buf", bufs=2) as s4_pool,
            tc.tile_pool(name="oc", bufs=2) as oc_pool,
            tc.tile_pool(name="psMain", bufs=6, space="PSUM") as psM_pool,
            tc.tile_pool(name="psL4", bufs=2, space="PSUM") as psL4_pool,
        ):
            # --- constants ---
            A1_s = consts.tile([128, 16 * 128], F16)
            nc.sync.dma_start(out=A1_s, in_=A1_d)
            W2e_s = consts.tile([128, 128], BF16)
            nc.sync.dma_start(out=W2e_s, in_=W2e_d)
            W2o_s = consts.tile([128, 128], BF16)
            nc.sync.dma_start(out=W2o_s, in_=W2o_d)
            W2eb_s = consts.tile([128, 128], BF16)
            nc.sync.dma_start(out=W2eb_s, in_=W2eb_d)
            W2ob_s = consts.tile([128, 128], BF16)
            nc.sync.dma_start(out=W2ob_s, in_=W2ob_d)
            W3a_s = consts.tile([128, 256], BF16)
            nc.sync.dma_start(out=W3a_s, in_=W3a_d)
            W3b_s = consts.tile([128, 256], BF16)
            nc.sync.dma_start(out=W3b_s, in_=W3b_d)
            W4_s = consts.tile([128, 6 * 128], BF16)
            nc.sync.dma_start(out=W4_s, in_=W4_d)
            Wf_s = consts.tile([128, 160], BF16)
            nc.sync.dma_start(out=Wf_s, in_=Wf_d)
            th1_s = consts.tile([128, 1], F32)
            nc.sync.dma_start(out=th1_s, in_=th1_d)
            nb1_s = consts.tile([128, 1], F32)
            nc.sync.dma_start(out=nb1_s, in_=nb1_d)
            th2_s = consts.tile([128, 1], F32)
            nc.sync.dma_start(out=th2_s, in_=th2_d)
            th3_s = consts.tile([128, 1], F32)
            nc.sync.dma_start(out=th3_s, in_=th3_d)
            b3_s = consts.tile([128, 1], F32)
            nc.sync.dma_start(out=b3_s, in_=b3_d)
            b4e_s = consts.tile([128, 1], F32)
            nc.sync.dma_start(out=b4e_s, in_=b4e_d)
            b4o_s = consts.tile([128, 1], F32)
            nc.sync.dma_start(out=b4o_s, in_=b4o_d)
            bf_s = consts.tile([10, 1], F32)
            nc.sync.dma_start(out=bf_s, in_=bf_d)

            # persistent double buffers (pads zeroed once)
            s1_bufs = []
            q_bufs = []
            s3_bufs = []
            s4_bufs = []
            for i in range(2):
                s1b = s1_pool.tile([128, 8 * BH], BF16, name=f"s1b{i}")
                s1_bufs.append(s1b)
                qb = q_pool.tile([128, 18 * BH], BF16, name=f"qb{i}")
                nc.vector.memset(qb[:, 0:BH], 0.0)
                nc.vector.memset(qb[:, 17 * BH:18 * BH], 0.0)
                q_bufs.append(qb)
                s3b = s3_pool.tile([128, 16 * BH], BF16, name=f"s3b{i}")
                s3_bufs.append(s3b)
                s4b = s4_pool.tile([128, 1024], BF16, name=f"s4b{i}")
                s4_bufs.append(s4b)

            for c in range(N_CHUNKS):
                s1t = s1_bufs[c % 2]
                qt = q_bufs[c % 2]
                s3t = s3_bufs[c % 2]
                s4t = s4_bufs[c % 2]
                csl = slice(c * BH, (c + 1) * BH)

                xh_t = xin_pool.tile([128, BH], F16, tag="xh")
                nc.sync.dma_start(out=xh_t, in_=xh_d[:, csl])
                xl_t = xin_pool.tile([128, BH], F16, tag="xl")
                nc.sync.dma_start(out=xl_t, in_=xl_d[:, csl])

                # ---- L1: 8 u-tiles, even/odd pairs, hi+lo accumulation ----
                for u in range(8):
                    psE = psM_pool.tile([128, BH], F32, tag="psM")
                    psO = psM_pool.tile([128, BH], F32, tag="psM")
                    se = slice((2 * u) * 128, (2 * u + 1) * 128)
                    so = slice((2 * u + 1) * 128, (2 * u + 2) * 128)
                    nc.tensor.matmul(psE, A1_s[:, se], xh_t, start=True, stop=False)
                    nc.tensor.matmul(psE, A1_s[:, se], xl_t, start=False, stop=True)
                    nc.tensor.matmul(psO, A1_s[:, so], xh_t, start=True, stop=False)
                    nc.tensor.matmul(psO, A1_s[:, so], xl_t, start=False, stop=True)
                    gE = gE_pool.tile([128, BH], BF16, tag="gE")
                    nc.scalar.activation(gE, psE, AF.Sign, bias=nb1_s)
                    gO = gO_pool.tile([128, BH], BF16, tag="gO")
                    nc.vector.tensor_scalar(out=gO, in0=psO, scalar1=th1_s,
                                            scalar2=0.5, op0=ALU.is_ge,
                                            op1=ALU.subtract)
                    nc.vector.scalar_tensor_tensor(
                        s1t[:, u * BH:(u + 1) * BH], gE, 0.5, gO,
                        op0=ALU.mult, op1=ALU.max)

                # ---- L2: 16 v-tiles (out positions 2v, 2v+1) --------------
                for v in range(16):
                    u = v // 2
                    usl = slice(u * BH, (u + 1) * BH)
                    ps2 = psM_pool.tile([128, BH], F32, tag="psM")
                    if v % 2 == 0:
                        nc.tensor.matmul(ps2, W2e_s, s1t[:, usl],
                                         start=True, stop=(u == 0))
                        if u > 0:
                            nc.tensor.matmul(
                                ps2, W2eb_s[96:128, :],
                                s1t[96:128, (u - 1) * BH:u * BH],
                                start=False, stop=True,
                                tile_position=(96, 0))
                    else:
                        nc.tensor.matmul(ps2, W2o_s, s1t[:, usl],
                                         start=True, stop=(u == 7))
                        if u < 7:
                            nc.tensor.matmul(
                                ps2, W2ob_s[0:32, :],
                                s1t[0:32, (u + 1) * BH:(u + 2) * BH],
                                start=False, stop=True)
                    nc.vector.tensor_scalar(
                        out=qt[:, (v + 1) * BH:(v + 2) * BH], in0=ps2,
                        scalar1=th2_s, scalar2=0.5,
                        op0=ALU.is_ge, op1=ALU.subtract)

                # ---- L3: 16 pooled-position tiles -------------------------
                for u in range(16):
                    mid = slice((u + 1) * BH, (u + 2) * BH)
                    p3a = psM_pool.tile([128, BH], F32, tag="psM")
                    p3b = psM_pool.tile([128, BH], F32, tag="psM")
                    nc.tensor.matmul(p3a, W3a_s[:, 0:128], qt[:, mid],
                                     start=True, stop=False)
                    nc.tensor.matmul(p3a, W3b_s[64:128, 0:128],
                                     qt[64:128, u * BH:(u + 1) * BH],
                                     start=False, stop=True)
                    nc.tensor.matmul(p3b, W3a_s[:, 128:256], qt[:, mid],
                                     start=True, stop=False)
                    nc.tensor.matmul(p3b, W3b_s[0:64, 128:256],
                                     qt[0:64, (u + 2) * BH:(u + 3) * BH],
                                     start=False, stop=True)
                    fa = fa_pool.tile([128, BH], BF16, tag="fa")
                    nc.scalar.activation(fa, p3a, AF.Sign, bias=b3_s)
                    fb = fb_pool.tile([128, BH], BF16, tag="fb")
                    nc.vector.tensor_scalar(out=fb, in0=p3b, scalar1=th3_s,
                                            scalar2=0.5, op0=ALU.is_ge,
                                            op1=ALU.subtract)
                    nc.vector.scalar_tensor_tensor(
                        s3t[:, u * BH:(u + 1) * BH], fa, 0.5, fb,
                        op0=ALU.mult, op1=ALU.max)

                # ---- L4: contract (ci,h), split by u-parity ---------------
                s3v = s3t.rearrange("p (u h b) -> p u h b", h=6, b=NB)
                for half in range(2):
                    ps4 = psL4_pool.tile([128, 512], F32, tag="ps4")
                    for h in range(6):
                        nc.tensor.matmul(
                            ps4, W4_s[:, 128 * h:128 * (h + 1)],
                            s3v[:, 8 * half:8 * (half + 1), h, :],
                            start=(h == 0), stop=(h == 5))
                    nc.scalar.activation(s4t[:, 512 * half:512 * (half + 1)],
                                         ps4, AF.Sign, bias=b4o_s)

                # ---- fc ---------------------------------------------------
                psf = psL4_pool.tile([10, 64], F32, tag="ps4")
                for w in range(16):
                    blk = w * 64
                    nc.tensor.matmul(psf, Wf_s[:, 10 * w:10 * (w + 1)],
                                     s4t[:, blk:blk + 64],
                                     start=(w == 0), stop=(w == 15))
                outc = oc_pool.tile([10, NB], F32)
                nc.vector.tensor_scalar_add(outc, psf, bf_s)
                nc.sync.dma_start(
                    out=y_d[c * NB:(c + 1) * NB, :].rearrange("b j -> j b"),
                    in_=outc)

    nc.compile()
    return nc


_PROGRAM = None


def _get_program():
    global _PROGRAM
    if _PROGRAM is None:
        _PROGRAM = build_program()
    return _PROGRAM


def run(trace=False, **inputs):
    inputs = {k: np.asarray(v) for k, v in inputs.items()}
    consts = prepare_host_tensors(
        **{k: inputs[k] for k in
           ("w1", "b1", "w2", "b2", "w3", "b3", "w4", "b4",
            "g1", "be1", "m1", "v1", "g2", "be2", "m2", "v2",
            "g3", "be3", "m3", "v3", "g4", "be4", "m4", "v4", "wf", "bf")})
    x = inputs["x"].astype(np.float32)           # [8192, 1, 6, 128]
    nc = _get_program()
    in_maps = []
    for k in range(N_CORES):
        xc = x[k * B_CORE:(k + 1) * B_CORE, 0]               # [1024, 6, 128]
        # chunk-column order (c, h, b) so L4's rhs slices are contiguous
        xT = np.ascontiguousarray(
            xc.reshape(N_CHUNKS, NB, 6, 128).transpose(0, 2, 1, 3)
            .reshape(B_CORE * 6, 128).T)
        xh = xT.astype(np.float16)
        xl = (xT - xh.astype(np.float32)).astype(np.float16)
        m = {"xh": xh, "xl": xl}
        m.update(consts)
        in_maps.append(m)
    res = run_bass_kernel_spmd(nc, in_maps, list(range(N_CORES)), trace=trace)
    y = np.concatenate([r["y"] for r in res.results], axis=0)
    return y.astype(np.float32), res


def kernel(**inputs):
    y, _ = run(trace=False, **inputs)
    return y


# revision 8
# speedup vs baseline: 1.9152x; 1.3024x over previous
"""Trainium2 Bass kernel for the binarized CNN (nn_CNN_binary_55001351193058).

Pure data-parallel over 8 NeuronCores (batch-sharded, 1024 samples/core).

Pipeline (per 64-sample chunk, all layouts (u, b, h)-column-major):
  - x is transposed + split hi/lo bf16 on the host; DMA'd as [128 w, cols].
  - L1: 8 u-tiles (4 pooled positions x 32ch rows), even/odd conv outputs as
    separate PSUM tiles, hi+lo bf16 matmuls sharing one stationary.
    Eviction: DVE tensor_scalar (>= th, -0.5) per half, GpSimd max -> s1 (+-0.5).
  - L2: shift-invariant even/odd stationaries + K=32 boundary matmuls;
    DVE tensor_scalar -> q (+-0.5), zero-padded u'=0/17 halo.
  - L3: 16 position-pair tiles; even u evicted via 2x ACT Sign (+-1),
    odd u via 2x DVE tensor_scalar (+-0.5); DVE max -> s3.
  - L4: contract (ci,h) split by u-parity (separate biases absorb the
    +-1 / +-0.5 scale), ACT Sign -> s4 (+-1).
  - fc: 16 accumulating matmuls, DVE bias add, DMA out.
Exact small-integer arithmetic in layers 2-4 + fc; BatchNorm+Hardtanh+binarize
folded into per-channel thresholds computed on the host in float64.
"""

import numpy as np
import ml_dtypes

import concourse.bass as bass
import concourse.mybir as mybir
import concourse.tile as tile
from concourse import bacc
from concourse.bass_utils import run_bass_kernel_spmd

F32 = mybir.dt.float32
F16 = mybir.dt.float16
BF16 = mybir.dt.bfloat16
AF = mybir.ActivationFunctionType
ALU = mybir.AluOpType

B_TOTAL = 8192
N_CORES = 8
B_CORE = B_TOTAL // N_CORES          # 1024
NB = 64                              # samples per chunk
N_CHUNKS = B_CORE // NB              # 16
BH = NB * 6                          # 384 (b,h) columns per chunk
EPS = 1e-5

bf16 = ml_dtypes.bfloat16


# ----------------------------------------------------------------------------
# Host-side weight preparation (float64 where it matters)
# ----------------------------------------------------------------------------

def _sgn(w):
    return np.where(w >= 0, 1.0, -1.0)


def _threshold(g, be, m, v, bias):
    inv = g.astype(np.float64) / np.sqrt(v.astype(np.float64) + EPS)
    assert (inv > 0).all(), "BN scale must be positive for threshold folding"
    sh = be.astype(np.float64) - m.astype(np.float64) * inv
    return -bias.astype(np.float64) - sh / inv


def _check_margin(th, grid_step, name):
    d = np.abs(th / grid_step - np.round(th / grid_step)) * grid_step
    if d.min() < 1e-4:
        raise AssertionError(f"threshold margin too small for {name}: {d.min()}")


def prepare_host_tensors(w1, b1, w2, b2, w3, b3, w4, b4,
                         g1, be1, m1, v1, g2, be2, m2, v2,
                         g3, be3, m3, v3, g4, be4, m4, v4, wf, bf):
    t1 = _threshold(g1, be1, m1, v1, b1)       # [32]
    t2 = _threshold(g2, be2, m2, v2, b2)       # [64]
    t3 = _threshold(g3, be3, m3, v3, b3)       # [128]
    t4 = _threshold(g4, be4, m4, v4, b4)       # [128]
    _check_margin(t2 / 2.0, 0.5, "t2")         # z2 ints when s1 = +-0.5
    _check_margin(t3, 2.0, "t3")               # z3 ints (q = +-0.5)
    _check_margin(t4 / 2.0, 1.0, "t4")

    s1 = _sgn(w1)[:, 0, 0, :].astype(np.float32)        # [32, 9]
    s2 = _sgn(w2)[:, :, 0, :].astype(np.float32)        # [64, 32, 3]
    s3 = _sgn(w3)[:, :, 0, :].astype(np.float32)        # [128, 64, 3]
    s4 = _sgn(w4)[:, :, :, 0].astype(np.float32)        # [128, 128, 6]
    sf = _sgn(wf).astype(np.float32)                    # [10, 2048]

    # L1: 16 m-tiles (8 u x even/odd), lhsT [w, (p,ci)].
    # row (p,ci) of tile (u,half) holds conv1 out at wy = 2*(4u+p)+half:
    #   wx = 2*wy + k - 4
    A1 = np.zeros((16, 128, 128), np.float32)
    for u in range(8):
        for half in range(2):
            m = 2 * u + half
            for p in range(4):
                wy = 2 * (4 * u + p) + half
                for k in range(9):
                    wx = 2 * wy + k - 4
                    if 0 <= wx < 128:
                        A1[m, wx, p * 32:(p + 1) * 32] = s1[:, k]
    A1 = A1.transpose(1, 0, 2).reshape(128, 16 * 128)

    # L2 stationaries, lhsT [(p,ci), (op,co)].
    # even v (out pos 4u+op):  k = p - op + 1
    # odd  v (out pos 4u+2+op): k = p - op - 1
    W2e = np.zeros((128, 128), np.float32)
    W2o = np.zeros((128, 128), np.float32)
    for p in range(4):
        for op in range(2):
            ke = p - op + 1
            if 0 <= ke <= 2:
                W2e[p * 32:(p + 1) * 32, op * 64:(op + 1) * 64] = s2[:, :, ke].T
            ko = p - op - 1
            if 0 <= ko <= 2:
                W2o[p * 32:(p + 1) * 32, op * 64:(op + 1) * 64] = s2[:, :, ko].T
    # boundary taps: even v op0 k0 from prev tile p3; odd v op1 k2 from next p0
    W2eb = np.zeros((128, 128), np.float32)
    W2eb[96:128, 0:64] = s2[:, :, 0].T
    W2ob = np.zeros((128, 128), np.float32)
    W2ob[0:32, 64:128] = s2[:, :, 2].T

    # L3 stationaries (as in the 2-op3-tile scheme), q rows (op, co2).
    W3a = np.zeros((128, 256), np.float32)
    W3a[0:64, 0:128] = s3[:, :, 1].T
    W3a[64:128, 0:128] = s3[:, :, 2].T
    W3a[0:64, 128:256] = s3[:, :, 0].T
    W3a[64:128, 128:256] = s3[:, :, 1].T
    W3b = np.zeros((128, 256), np.float32)
    W3b[64:128, 0:128] = s3[:, :, 0].T     # even tile bnd: odd[u-1], k=0
    W3b[0:64, 128:256] = s3[:, :, 2].T     # odd tile bnd: even[u+1], k=2

    W4t = s4.transpose(2, 1, 0).reshape(6, 128, 128)    # [h][ci, co]
    W4t = W4t.transpose(1, 0, 2).reshape(128, 6 * 128)  # [ci, (h,co)]

    Wf = sf.reshape(10, 128, 16)                         # [j, co, w]
    Wf = Wf.transpose(1, 2, 0).reshape(128, 160)         # [co, (w,j)]

    th1 = np.tile(t1, 4).astype(np.float32).reshape(128, 1)          # (p,ci)
    nb1 = (-th1).astype(np.float32)                                  # ACT bias
    b2v = (-np.concatenate([t2, t2]) / 2.0).astype(np.float32).reshape(128, 1)
    th3v = t3.astype(np.float32).reshape(128, 1)       # q = +-1 -> z3 full scale
    b3v = (-t3).astype(np.float32).reshape(128, 1)
    b4e = (-t4).astype(np.float32).reshape(128, 1)                   # s3 = +-1
    b4o = (-t4 / 2.0).astype(np.float32).reshape(128, 1)             # s3 = +-0.5
    bfv = bf.astype(np.float32).reshape(10, 1)

    return dict(
        A1=A1.astype(np.float16),
        W2e=W2e.astype(bf16), W2o=W2o.astype(bf16),
        W2eb=W2eb.astype(bf16), W2ob=W2ob.astype(bf16),
        W3a=W3a.astype(bf16), W3b=W3b.astype(bf16),
        W4t=W4t.astype(bf16), Wf=Wf.astype(bf16),
        th1=th1, nb1=nb1, b2v=b2v, th3v=th3v, b3v=b3v, b4e=b4e, b4o=b4o,
        bfv=bfv,
    )


# ----------------------------------------------------------------------------
# Bass program (identical SPMD program for each core)
# ----------------------------------------------------------------------------

def build_program():
    nc = bacc.Bacc("TRN2", target_bir_lowering=False, debug=False)

    xh_d = nc.dram_tensor("xh", [128, B_CORE * 6], F16, kind="ExternalInput").ap()
    xl_d = nc.dram_tensor("xl", [128, B_CORE * 6], F16, kind="ExternalInput").ap()
    A1_d = nc.dram_tensor("A1", [128, 16 * 128], F16, kind="ExternalInput").ap()
    W2e_d = nc.dram_tensor("W2e", [128, 128], BF16, kind="ExternalInput").ap()
    W2o_d = nc.dram_tensor("W2o", [128, 128], BF16, kind="ExternalInput").ap()
    W2eb_d = nc.dram_tensor("W2eb", [128, 128], BF16, kind="ExternalInput").ap()
    W2ob_d = nc.dram_tensor("W2ob", [128, 128], BF16, kind="ExternalInput").ap()
    W3a_d = nc.dram_tensor("W3a", [128, 256], BF16, kind="ExternalInput").ap()
    W3b_d = nc.dram_tensor("W3b", [128, 256], BF16, kind="ExternalInput").ap()
    W4_d = nc.dram_tensor("W4t", [128, 6 * 128], BF16, kind="ExternalInput").ap()
    Wf_d = nc.dram_tensor("Wf", [128, 160], BF16, kind="ExternalInput").ap()
    th1_d = nc.dram_tensor("th1", [128, 1], F32, kind="ExternalInput").ap()
    nb1_d = nc.dram_tensor("nb1", [128, 1], F32, kind="ExternalInput").ap()
    th2_d = nc.dram_tensor("b2v", [128, 1], F32, kind="ExternalInput").ap()
    th3_d = nc.dram_tensor("th3v", [128, 1], F32, kind="ExternalInput").ap()
    b3_d = nc.dram_tensor("b3v", [128, 1], F32, kind="ExternalInput").ap()
    b4e_d = nc.dram_tensor("b4e", [128, 1], F32, kind="ExternalInput").ap()
    b4o_d = nc.dram_tensor("b4o", [128, 1], F32, kind="ExternalInput").ap()
    bf_d = nc.dram_tensor("bfv", [10, 1], F32, kind="ExternalInput").ap()

    y_d = nc.dram_tensor("y", [B_CORE, 10], F32, kind="ExternalOutput").ap()

    with tile.TileContext(nc) as tc:
        with (
            tc.tile_pool(name="consts", bufs=1) as consts,
            tc.tile_pool(name="xin", bufs=4) as xin_pool,
            tc.tile_pool(name="greadyE", bufs=3) as gE_pool,
            tc.tile_pool(name="greadyO", bufs=3) as gO_pool,
            tc.tile_pool(name="s1buf", bufs=2) as s1_pool,
            tc.tile_pool(name="qbuf", bufs=2) as q_pool,
            tc.tile_pool(name="fa", bufs=3) as fa_pool,
            tc.tile_pool(name="fb", bufs=3) as fb_pool,
            tc.tile_pool(name="s3buf", bufs=2) as s3_pool,
            tc.tile_pool(name="s4buf", bufs=2) as s4_pool,
            tc.tile_pool(name="oc", bufs=2) as oc_pool,
            tc.tile_pool(name="psMain", bufs=6, space="PSUM") as psM_pool,
            tc.tile_pool(name="psL4", bufs=2, space="PSUM") as psL4_pool,
        ):
            # --- constants ---
            A1_s = consts.tile([128, 16 * 128], F16)
            nc.sync.dma_start(out=A1_s, in_=A1_d)
            W2e_s = consts.tile([128, 128], BF16)
            nc.sync.dma_start(out=W2e_s, in_=W2e_d)
            W2o_s = consts.tile([128, 128], BF16)
            nc.sync.dma_start(out=W2o_s, in_=W2o_d)
            W2eb_s = consts.tile([128, 128], BF16)
            nc.sync.dma_start(out=W2eb_s, in_=W2eb_d)
            W2ob_s = consts.tile([128, 128], BF16)
            nc.sync.dma_start(out=W2ob_s, in_=W2ob_d)
            W3a_s = consts.tile([128, 256], BF16)
            nc.sync.dma_start(out=W3a_s, in_=W3a_d)
            W3b_s = consts.tile([128, 256], BF16)
            nc.sync.dma_start(out=W3b_s, in_=W3b_d)
            W4_s = consts.tile([128, 6 * 128], BF16)
            nc.sync.dma_start(out=W4_s, in_=W4_d)
            Wf_s = consts.tile([128, 160], BF16)
            nc.sync.dma_start(out=Wf_s, in_=Wf_d)
            th1_s = consts.tile([128, 1], F32)
            nc.sync.dma_start(out=th1_s, in_=th1_d)
            nb1_s = consts.tile([128, 1], F32)
            nc.sync.dma_start(out=nb1_s, in_=nb1_d)
            th2_s = consts.tile([128, 1], F32)
            nc.sync.dma_start(out=th2_s, in_=th2_d)
            th3_s = consts.tile([128, 1], F32)
            nc.sync.dma_start(out=th3_s, in_=th3_d)
            b3_s = consts.tile([128, 1], F32)
            nc.sync.dma_start(out=b3_s, in_=b3_d)
            b4e_s = consts.tile([128, 1], F32)
            nc.sync.dma_start(out=b4e_s, in_=b4e_d)
            b4o_s = consts.tile([128, 1], F32)
            nc.sync.dma_start(out=b4o_s, in_=b4o_d)
            bf_s = consts.tile([10, 1], F32)
            nc.sync.dma_start(out=bf_s, in_=bf_d)

            # persistent double buffers (pads zeroed once)
            s1_bufs = []
            q_bufs = []
            s3_bufs = []
            s4_bufs = []
            for i in range(2):
                s1b = s1_pool.tile([128, 8 * BH], BF16, name=f"s1b{i}")
                s1_bufs.append(s1b)
                qb = q_pool.tile([128, 18 * BH], BF16, name=f"qb{i}")
                nc.vector.memset(qb[:, 0:BH], 0.0)
                nc.vector.memset(qb[:, 17 * BH:18 * BH], 0.0)
                q_bufs.append(qb)
                s3b = s3_pool.tile([128, 16 * BH], BF16, name=f"s3b{i}")
                s3_bufs.append(s3b)
                s4b = s4_pool.tile([128, 1024], BF16, name=f"s4b{i}")
                s4_bufs.append(s4b)

            for c in range(N_CHUNKS):
                s1t = s1_bufs[c % 2]
                qt = q_bufs[c % 2]
                s3t = s3_bufs[c % 2]
                s4t = s4_bufs[c % 2]
                csl = slice(c * BH, (c + 1) * BH)

                xh_t = xin_pool.tile([128, BH], F16, tag="xh")
                nc.sync.dma_start(out=xh_t, in_=xh_d[:, csl])
                xl_t = xin_pool.tile([128, BH], F16, tag="xl")
                nc.sync.dma_start(out=xl_t, in_=xl_d[:, csl])

                # ---- L1: 8 u-tiles, even/odd pairs, hi+lo accumulation ----
                for u in range(8):
                    psE = psM_pool.tile([128, BH], F32, tag="psM")
                    psO = psM_pool.tile([128, BH], F32, tag="psM")
                    se = slice((2 * u) * 128, (2 * u + 1) * 128)
                    so = slice((2 * u + 1) * 128, (2 * u + 2) * 128)
                    nc.tensor.matmul(psE, A1_s[:, se], xh_t, start=True, stop=False)
                    nc.tensor.matmul(psE, A1_s[:, se], xl_t, start=False, stop=True)
                    nc.tensor.matmul(psO, A1_s[:, so], xh_t, start=True, stop=False)
                    nc.tensor.matmul(psO, A1_s[:, so], xl_t, start=False, stop=True)
                    gE = gE_pool.tile([128, BH], BF16, tag="gE")
                    nc.scalar.activation(gE, psE, AF.Sign, bias=nb1_s)
                    gO = gO_pool.tile([128, BH], BF16, tag="gO")
                    nc.vector.tensor_scalar(out=gO, in0=psO, scalar1=th1_s,
                                            scalar2=0.5, op0=ALU.is_ge,
                                            op1=ALU.subtract)
                    nc.vector.scalar_tensor_tensor(
                        s1t[:, u * BH:(u + 1) * BH], gE, 0.5, gO,
                        op0=ALU.mult, op1=ALU.max)

                # ---- L2: 16 v-tiles (out positions 2v, 2v+1) --------------
                for v in range(16):
                    u = v // 2
                    usl = slice(u * BH, (u + 1) * BH)
                    ps2 = psM_pool.tile([128, BH], F32, tag="psM")
                    if v % 2 == 0:
                        nc.tensor.matmul(ps2, W2e_s, s1t[:, usl],
                                         start=True, stop=(u == 0))
                        if u > 0:
                            nc.tensor.matmul(
                                ps2, W2eb_s[96:128, :],
                                s1t[96:128, (u - 1) * BH:u * BH],
                                start=False, stop=True,
                                tile_position=(96, 0))
                    else:
                        nc.tensor.matmul(ps2, W2o_s, s1t[:, usl],
                                         start=True, stop=(u == 7))
                        if u < 7:
                            nc.tensor.matmul(
                                ps2, W2ob_s[0:32, :],
                                s1t[0:32, (u + 1) * BH:(u + 2) * BH],
                                start=False, stop=True)
                    nc.scalar.activation(qt[:, (v + 1) * BH:(v + 2) * BH],
                                         ps2, AF.Sign, bias=th2_s)

                # ---- L3: 16 pooled-position tiles -------------------------
                for u in range(16):
                    mid = slice((u + 1) * BH, (u + 2) * BH)
                    p3a = psM_pool.tile([128, BH], F32, tag="psM")
                    p3b = psM_pool.tile([128, BH], F32, tag="psM")
                    nc.tensor.matmul(p3a, W3a_s[:, 0:128], qt[:, mid],
                                     start=True, stop=False)
                    nc.tensor.matmul(p3a, W3b_s[64:128, 0:128],
                                     qt[64:128, u * BH:(u + 1) * BH],
                                     start=False, stop=True)
                    nc.tensor.matmul(p3b, W3a_s[:, 128:256], qt[:, mid],
                                     start=True, stop=False)
                    nc.tensor.matmul(p3b, W3b_s[0:64, 128:256],
                                     qt[0:64, (u + 2) * BH:(u + 3) * BH],
                                     start=False, stop=True)
                    fa = fa_pool.tile([128, BH], BF16, tag="fa")
                    nc.scalar.activation(fa, p3a, AF.Sign, bias=b3_s)
                    fb = fb_pool.tile([128, BH], BF16, tag="fb")
                    nc.vector.tensor_scalar(out=fb, in0=p3b, scalar1=th3_s,
                                            scalar2=0.5, op0=ALU.is_ge,
                                            op1=ALU.subtract)
                    nc.vector.scalar_tensor_tensor(
                        s3t[:, u * BH:(u + 1) * BH], fa, 0.5, fb,
                        op0=ALU.mult, op1=ALU.max)

                # ---- L4: contract (ci,h), split by u-parity ---------------
                s3v = s3t.rearrange("p (u h b) -> p u h b", h=6, b=NB)
                for half in range(2):
                    ps4 = psL4_pool.tile([128, 512], F32, tag="ps4")
                    for h in range(6):
                        nc.tensor.matmul(
                            ps4, W4_s[:, 128 * h:128 * (h + 1)],
                            s3v[:, 8 * half:8 * (half + 1), h, :],
                            start=(h == 0), stop=(h == 5))
                    nc.scalar.activation(s4t[:, 512 * half:512 * (half + 1)],
                                         ps4, AF.Sign, bias=b4o_s)

                # ---- fc ---------------------------------------------------
                psf = psL4_pool.tile([10, 64], F32, tag="ps4")
                for w in range(16):
                    blk = w * 64
                    nc.tensor.matmul(psf, Wf_s[:, 10 * w:10 * (w + 1)],
                                     s4t[:, blk:blk + 64],
                                     start=(w == 0), stop=(w == 15))
                outc = oc_pool.tile([10, NB], F32)
                nc.vector.tensor_scalar_add(outc, psf, bf_s)
                nc.sync.dma_start(
                    out=y_d[c * NB:(c + 1) * NB, :].rearrange("b j -> j b"),
                    in_=outc)

    nc.compile()
    return nc


_PROGRAM = None


def _get_program():
    global _PROGRAM
    if _PROGRAM is None:
        _PROGRAM = build_program()
    return _PROGRAM


def run(trace=False, **inputs):
    inputs = {k: np.asarray(v) for k, v in inputs.items()}
    consts = prepare_host_tensors(
        **{k: inputs[k] for k in
           ("w1", "b1", "w2", "b2", "w3", "b3", "w4", "b4",
            "g1", "be1", "m1", "v1", "g2", "be2", "m2", "v2",
            "g3", "be3", "m3", "v3", "g4", "be4", "m4", "v4", "wf", "bf")})
    x = inputs["x"].astype(np.float32)           # [8192, 1, 6, 128]
    nc = _get_program()
    in_maps = []
    for k in range(N_CORES):
        xc = x[k * B_CORE:(k + 1) * B_CORE, 0]               # [1024, 6, 128]
        # chunk-column order (c, h, b) so L4's rhs slices are contiguous
        xT = np.ascontiguousarray(
            xc.reshape(N_CHUNKS, NB, 6, 128).transpose(0, 2, 1, 3)
            .reshape(B_CORE * 6, 128).T)
        xh = xT.astype(np.float16)
        xl = (xT - xh.astype(np.float32)).astype(np.float16)
        m = {"xh": xh, "xl": xl}
        m.update(consts)
        in_maps.append(m)
    res = run_bass_kernel_spmd(nc, in_maps, list(range(N_CORES)), trace=trace)
    y = np.concatenate([r["y"] for r in res.results], axis=0)
    return y.astype(np.float32), res


def kernel(**inputs):
    y, _ = run(trace=False, **inputs)
    return y


# revision 9
# speedup vs baseline: 2.2513x; 1.1755x over previous
"""Trainium2 Bass kernel for the binarized CNN (nn_CNN_binary_55001351193058).

Pure data-parallel over 8 NeuronCores (batch-sharded, 1024 samples/core).

Pipeline (per 64-sample chunk, all layouts (u, b, h)-column-major):
  - x is transposed + split hi/lo bf16 on the host; DMA'd as [128 w, cols].
  - L1: 8 u-tiles (4 pooled positions x 32ch rows), even/odd conv outputs as
    separate PSUM tiles, hi+lo bf16 matmuls sharing one stationary.
    Eviction: DVE tensor_scalar (>= th, -0.5) per half, GpSimd max -> s1 (+-0.5).
  - L2: shift-invariant even/odd stationaries + K=32 boundary matmuls;
    DVE tensor_scalar -> q (+-0.5), zero-padded u'=0/17 halo.
  - L3: 16 position-pair tiles; even u evicted via 2x ACT Sign (+-1),
    odd u via 2x DVE tensor_scalar (+-0.5); DVE max -> s3.
  - L4: contract (ci,h) split by u-parity (separate biases absorb the
    +-1 / +-0.5 scale), ACT Sign -> s4 (+-1).
  - fc: 16 accumulating matmuls, DVE bias add, DMA out.
Exact small-integer arithmetic in layers 2-4 + fc; BatchNorm+Hardtanh+binarize
folded into per-channel thresholds computed on the host in float64.
"""

import numpy as np
import ml_dtypes

import concourse.bass as bass
import concourse.mybir as mybir
import concourse.tile as tile
from concourse import bacc
from concourse.bass_utils import run_bass_kernel_spmd

F32 = mybir.dt.float32
F16 = mybir.dt.float16
BF16 = mybir.dt.bfloat16
AF = mybir.ActivationFunctionType
ALU = mybir.AluOpType

B_TOTAL = 8192
N_CORES = 8
B_CORE = B_TOTAL // N_CORES          # 1024
NB = 64                              # samples per chunk
N_CHUNKS = B_CORE // NB              # 16
BH = NB * 6                          # 384 (b,h) columns per chunk
EPS = 1e-5

bf16 = ml_dtypes.bfloat16


# ----------------------------------------------------------------------------
# Host-side weight preparation (float64 where it matters)
# ----------------------------------------------------------------------------

def _sgn(w):
    return np.where(w >= 0, 1.0, -1.0)


def _threshold(g, be, m, v, bias):
    inv = g.astype(np.float64) / np.sqrt(v.astype(np.float64) + EPS)
    assert (inv > 0).all(), "BN scale must be positive for threshold folding"
    sh = be.astype(np.float64) - m.astype(np.float64) * inv
    return -bias.astype(np.float64) - sh / inv


def _check_margin(th, grid_step, name):
    d = np.abs(th / grid_step - np.round(th / grid_step)) * grid_step
    if d.min() < 1e-4:
        raise AssertionError(f"threshold margin too small for {name}: {d.min()}")


def prepare_host_tensors(w1, b1, w2, b2, w3, b3, w4, b4,
                         g1, be1, m1, v1, g2, be2, m2, v2,
                         g3, be3, m3, v3, g4, be4, m4, v4, wf, bf):
    t1 = _threshold(g1, be1, m1, v1, b1)       # [32]
    t2 = _threshold(g2, be2, m2, v2, b2)       # [64]
    t3 = _threshold(g3, be3, m3, v3, b3)       # [128]
    t4 = _threshold(g4, be4, m4, v4, b4)       # [128]
    _check_margin(t2 / 2.0, 0.5, "t2")         # z2 ints when s1 = +-0.5
    _check_margin(t3, 2.0, "t3")               # z3 ints (q = +-0.5)
    _check_margin(t4 / 2.0, 1.0, "t4")

    s1 = _sgn(w1)[:, 0, 0, :].astype(np.float32)        # [32, 9]
    s2 = _sgn(w2)[:, :, 0, :].astype(np.float32)        # [64, 32, 3]
    s3 = _sgn(w3)[:, :, 0, :].astype(np.float32)        # [128, 64, 3]
    s4 = _sgn(w4)[:, :, :, 0].astype(np.float32)        # [128, 128, 6]
    sf = _sgn(wf).astype(np.float32)                    # [10, 2048]

    # L1: 16 m-tiles (8 u x even/odd), lhsT [w, (p,ci)].
    # row (p,ci) of tile (u,half) holds conv1 out at wy = 2*(4u+p)+half:
    #   wx = 2*wy + k - 4
    A1 = np.zeros((16, 128, 128), np.float32)
    for u in range(8):
        for half in range(2):
            m = 2 * u + half
            for p in range(4):
                wy = 2 * (4 * u + p) + half
                for k in range(9):
                    wx = 2 * wy + k - 4
                    if 0 <= wx < 128:
                        A1[m, wx, p * 32:(p + 1) * 32] = s1[:, k]
    A1 = A1.transpose(1, 0, 2).reshape(128, 16 * 128)

    # L2 stationaries, lhsT [(p,ci), (op,co)].
    # even v (out pos 4u+op):  k = p - op + 1
    # odd  v (out pos 4u+2+op): k = p - op - 1
    W2e = np.zeros((128, 128), np.float32)
    W2o = np.zeros((128, 128), np.float32)
    for p in range(4):
        for op in range(2):
            ke = p - op + 1
            if 0 <= ke <= 2:
                W2e[p * 32:(p + 1) * 32, op * 64:(op + 1) * 64] = s2[:, :, ke].T
            ko = p - op - 1
            if 0 <= ko <= 2:
                W2o[p * 32:(p + 1) * 32, op * 64:(op + 1) * 64] = s2[:, :, ko].T
    # boundary taps: even v op0 k0 from prev tile p3; odd v op1 k2 from next p0
    W2eb = np.zeros((128, 128), np.float32)
    W2eb[96:128, 0:64] = s2[:, :, 0].T
    W2ob = np.zeros((128, 128), np.float32)
    W2ob[0:32, 64:128] = s2[:, :, 2].T

    # L3 stationaries (as in the 2-op3-tile scheme), q rows (op, co2).
    W3a = np.zeros((128, 256), np.float32)
    W3a[0:64, 0:128] = s3[:, :, 1].T
    W3a[64:128, 0:128] = s3[:, :, 2].T
    W3a[0:64, 128:256] = s3[:, :, 0].T
    W3a[64:128, 128:256] = s3[:, :, 1].T
    W3b = np.zeros((128, 256), np.float32)
    W3b[64:128, 0:128] = s3[:, :, 0].T     # even tile bnd: odd[u-1], k=0
    W3b[0:64, 128:256] = s3[:, :, 2].T     # odd tile bnd: even[u+1], k=2

    W4t = s4.transpose(2, 1, 0).reshape(6, 128, 128)    # [h][ci, co]
    W4t = W4t.transpose(1, 0, 2).reshape(128, 6 * 128)  # [ci, (h,co)]

    Wf = sf.reshape(10, 128, 16)                         # [j, co, w]
    Wf = Wf.transpose(1, 2, 0).reshape(128, 160)         # [co, (w,j)]

    th1 = np.tile(t1, 4).astype(np.float32).reshape(128, 1)          # (p,ci)
    nb1 = (-th1).astype(np.float32)                                  # ACT bias
    b2v = (-np.concatenate([t2, t2]) / 2.0).astype(np.float32).reshape(128, 1)
    th3v = t3.astype(np.float32).reshape(128, 1)       # q = +-1 -> z3 full scale
    b3v = (-t3).astype(np.float32).reshape(128, 1)
    b4e = (-t4).astype(np.float32).reshape(128, 1)                   # s3 = +-1
    b4o = (-t4 / 2.0).astype(np.float32).reshape(128, 1)             # s3 = +-0.5
    bfv = bf.astype(np.float32).reshape(10, 1)

    return dict(
        A1=A1.astype(np.float16),
        W2e=W2e.astype(bf16), W2o=W2o.astype(bf16),
        W2eb=W2eb.astype(bf16), W2ob=W2ob.astype(bf16),
        W3a=W3a.astype(bf16), W3b=W3b.astype(bf16),
        W4t=W4t.astype(bf16), Wf=Wf.astype(bf16),
        th1=th1, nb1=nb1, b2v=b2v, th3v=th3v, b3v=b3v, b4e=b4e, b4o=b4o,
        bfv=bfv,
    )


# ----------------------------------------------------------------------------
# Bass program (identical SPMD program for each core)
# ----------------------------------------------------------------------------

def build_program():
    nc = bacc.Bacc("TRN2", target_bir_lowering=False, debug=False)

    xh_d = nc.dram_tensor("xh", [128, B_CORE * 6], F16, kind="ExternalInput").ap()
    xl_d = nc.dram_tensor("xl", [128, B_CORE * 6], F16, kind="ExternalInput").ap()
    A1_d = nc.dram_tensor("A1", [128, 16 * 128], F16, kind="ExternalInput").ap()
    W2e_d = nc.dram_tensor("W2e", [128, 128], BF16, kind="ExternalInput").ap()
    W2o_d = nc.dram_tensor("W2o", [128, 128], BF16, kind="ExternalInput").ap()
    W2eb_d = nc.dram_tensor("W2eb", [128, 128], BF16, kind="ExternalInput").ap()
    W2ob_d = nc.dram_tensor("W2ob", [128, 128], BF16, kind="ExternalInput").ap()
    W3a_d = nc.dram_tensor("W3a", [128, 256], BF16, kind="ExternalInput").ap()
    W3b_d = nc.dram_tensor("W3b", [128, 256], BF16, kind="ExternalInput").ap()
    W4_d = nc.dram_tensor("W4t", [128, 6 * 128], BF16, kind="ExternalInput").ap()
    Wf_d = nc.dram_tensor("Wf", [128, 160], BF16, kind="ExternalInput").ap()
    th1_d = nc.dram_tensor("th1", [128, 1], F32, kind="ExternalInput").ap()
    nb1_d = nc.dram_tensor("nb1", [128, 1], F32, kind="ExternalInput").ap()
    th2_d = nc.dram_tensor("b2v", [128, 1], F32, kind="ExternalInput").ap()
    th3_d = nc.dram_tensor("th3v", [128, 1], F32, kind="ExternalInput").ap()
    b3_d = nc.dram_tensor("b3v", [128, 1], F32, kind="ExternalInput").ap()
    b4e_d = nc.dram_tensor("b4e", [128, 1], F32, kind="ExternalInput").ap()
    b4o_d = nc.dram_tensor("b4o", [128, 1], F32, kind="ExternalInput").ap()
    bf_d = nc.dram_tensor("bfv", [10, 1], F32, kind="ExternalInput").ap()

    y_d = nc.dram_tensor("y", [B_CORE, 10], F32, kind="ExternalOutput").ap()

    with tile.TileContext(nc) as tc:
        with (
            tc.tile_pool(name="consts", bufs=1) as consts,
            tc.tile_pool(name="xin", bufs=4) as xin_pool,
            tc.tile_pool(name="greadyE", bufs=3) as gE_pool,
            tc.tile_pool(name="greadyO", bufs=3) as gO_pool,
            tc.tile_pool(name="s1buf", bufs=2) as s1_pool,
            tc.tile_pool(name="qbuf", bufs=2) as q_pool,
            tc.tile_pool(name="fa", bufs=3) as fa_pool,
            tc.tile_pool(name="fb", bufs=3) as fb_pool,
            tc.tile_pool(name="s3buf", bufs=2) as s3_pool,
            tc.tile_pool(name="s4buf", bufs=2) as s4_pool,
            tc.tile_pool(name="oc", bufs=2) as oc_pool,
            tc.tile_pool(name="psMain", bufs=6, space="PSUM") as psM_pool,
            tc.tile_pool(name="psL4", bufs=2, space="PSUM") as psL4_pool,
        ):
            # --- constants ---
            A1_s = consts.tile([128, 16 * 128], F16)
            nc.sync.dma_start(out=A1_s, in_=A1_d)
            W2e_s = consts.tile([128, 128], BF16)
            nc.sync.dma_start(out=W2e_s, in_=W2e_d)
            W2o_s = consts.tile([128, 128], BF16)
            nc.sync.dma_start(out=W2o_s, in_=W2o_d)
            W2eb_s = consts.tile([128, 128], BF16)
            nc.sync.dma_start(out=W2eb_s, in_=W2eb_d)
            W2ob_s = consts.tile([128, 128], BF16)
            nc.sync.dma_start(out=W2ob_s, in_=W2ob_d)
            W3a_s = consts.tile([128, 256], BF16)
            nc.sync.dma_start(out=W3a_s, in_=W3a_d)
            W3b_s = consts.tile([128, 256], BF16)
            nc.sync.dma_start(out=W3b_s, in_=W3b_d)
            W4_s = consts.tile([128, 6 * 128], BF16)
            nc.sync.dma_start(out=W4_s, in_=W4_d)
            Wf_s = consts.tile([128, 160], BF16)
            nc.sync.dma_start(out=Wf_s, in_=Wf_d)
            th1_s = consts.tile([128, 1], F32)
            nc.sync.dma_start(out=th1_s, in_=th1_d)
            nb1_s = consts.tile([128, 1], F32)
            nc.sync.dma_start(out=nb1_s, in_=nb1_d)
            th2_s = consts.tile([128, 1], F32)
            nc.sync.dma_start(out=th2_s, in_=th2_d)
            th3_s = consts.tile([128, 1], F32)
            nc.sync.dma_start(out=th3_s, in_=th3_d)
            b3_s = consts.tile([128, 1], F32)
            nc.sync.dma_start(out=b3_s, in_=b3_d)
            b4e_s = consts.tile([128, 1], F32)
            nc.sync.dma_start(out=b4e_s, in_=b4e_d)
            b4o_s = consts.tile([128, 1], F32)
            nc.sync.dma_start(out=b4o_s, in_=b4o_d)
            bf_s = consts.tile([10, 1], F32)
            nc.sync.dma_start(out=bf_s, in_=bf_d)

            # persistent double buffers (pads zeroed once)
            s1_bufs = []
            q_bufs = []
            s3_bufs = []
            s4_bufs = []
            for i in range(2):
                s1b = s1_pool.tile([128, 8 * BH], BF16, name=f"s1b{i}")
                s1_bufs.append(s1b)
                qb = q_pool.tile([128, 18 * BH], BF16, name=f"qb{i}")
                nc.vector.memset(qb[:, 0:BH], 0.0)
                nc.vector.memset(qb[:, 17 * BH:18 * BH], 0.0)
                q_bufs.append(qb)
                s3b = s3_pool.tile([128, 16 * BH], BF16, name=f"s3b{i}")
                s3_bufs.append(s3b)
                s4b = s4_pool.tile([128, 1024], BF16, name=f"s4b{i}")
                s4_bufs.append(s4b)

            for c in range(N_CHUNKS):
                s1t = s1_bufs[c % 2]
                qt = q_bufs[c % 2]
                s3t = s3_bufs[c % 2]
                s4t = s4_bufs[c % 2]
                csl = slice(c * BH, (c + 1) * BH)

                xh_t = xin_pool.tile([128, BH], F16, tag="xh")
                nc.sync.dma_start(out=xh_t, in_=xh_d[:, csl])
                xl_t = xin_pool.tile([128, BH], F16, tag="xl")
                nc.sync.dma_start(out=xl_t, in_=xl_d[:, csl])

                # ---- L1: 8 u-tiles, even/odd pairs, hi+lo accumulation ----
                for u in range(8):
                    psE = psM_pool.tile([128, BH], F32, tag="psM")
                    psO = psM_pool.tile([128, BH], F32, tag="psM")
                    se = slice((2 * u) * 128, (2 * u + 1) * 128)
                    so = slice((2 * u + 1) * 128, (2 * u + 2) * 128)
                    nc.tensor.matmul(psE, A1_s[:, se], xh_t, start=True, stop=False)
                    nc.tensor.matmul(psE, A1_s[:, se], xl_t, start=False, stop=True)
                    nc.tensor.matmul(psO, A1_s[:, so], xh_t, start=True, stop=False)
                    nc.tensor.matmul(psO, A1_s[:, so], xl_t, start=False, stop=True)
                    gE = gE_pool.tile([128, BH], BF16, tag="gE")
                    nc.scalar.activation(gE, psE, AF.Sign, bias=nb1_s)
                    gO = gO_pool.tile([128, BH], BF16, tag="gO")
                    nc.vector.tensor_scalar(out=gO, in0=psO, scalar1=th1_s,
                                            scalar2=0.5, op0=ALU.is_ge,
                                            op1=ALU.subtract)
                    nc.vector.scalar_tensor_tensor(
                        s1t[:, u * BH:(u + 1) * BH], gE, 0.5, gO,
                        op0=ALU.mult, op1=ALU.max)

                # ---- L2: 16 v-tiles (out positions 2v, 2v+1) --------------
                for uu in range(0, 8, 2):
                    pse = [psM_pool.tile([128, BH], F32, tag="psM",
                                         name=f"ps2_{uu}_{i}") for i in range(4)]
                    for i, u in enumerate((uu, uu + 1)):
                        usl = slice(u * BH, (u + 1) * BH)
                        nc.tensor.matmul(pse[2 * i], W2e_s, s1t[:, usl],
                                         start=True, stop=(u == 0))
                    for i, u in enumerate((uu, uu + 1)):
                        usl = slice(u * BH, (u + 1) * BH)
                        nc.tensor.matmul(pse[2 * i + 1], W2o_s, s1t[:, usl],
                                         start=True, stop=(u == 7))
                    for i, u in enumerate((uu, uu + 1)):
                        if u > 0:
                            nc.tensor.matmul(
                                pse[2 * i], W2eb_s[96:128, :],
                                s1t[96:128, (u - 1) * BH:u * BH],
                                start=False, stop=True,
                                tile_position=(96, 0))
                    for i, u in enumerate((uu, uu + 1)):
                        if u < 7:
                            nc.tensor.matmul(
                                pse[2 * i + 1], W2ob_s[0:32, :],
                                s1t[0:32, (u + 1) * BH:(u + 2) * BH],
                                start=False, stop=True)
                    for i, u in enumerate((uu, uu + 1)):
                        for j, ps2 in ((0, pse[2 * i]), (1, pse[2 * i + 1])):
                            v = 2 * u + j
                            nc.scalar.activation(
                                qt[:, (v + 1) * BH:(v + 2) * BH],
                                ps2, AF.Sign, bias=th2_s)

                # ---- L3: 16 pooled-position tiles -------------------------
                for jj in range(0, 16, 2):
                    ps3 = [psM_pool.tile([128, BH], F32, tag="psM",
                                         name=f"p3_{jj}_{i}") for i in range(4)]
                    for i, u in enumerate((jj, jj + 1)):
                        mid = slice((u + 1) * BH, (u + 2) * BH)
                        nc.tensor.matmul(ps3[2 * i], W3a_s[:, 0:128],
                                         qt[:, mid], start=True, stop=False)
                    for i, u in enumerate((jj, jj + 1)):
                        mid = slice((u + 1) * BH, (u + 2) * BH)
                        nc.tensor.matmul(ps3[2 * i + 1], W3a_s[:, 128:256],
                                         qt[:, mid], start=True, stop=False)
                    for i, u in enumerate((jj, jj + 1)):
                        nc.tensor.matmul(ps3[2 * i], W3b_s[64:128, 0:128],
                                         qt[64:128, u * BH:(u + 1) * BH],
                                         start=False, stop=True)
                    for i, u in enumerate((jj, jj + 1)):
                        nc.tensor.matmul(ps3[2 * i + 1], W3b_s[0:64, 128:256],
                                         qt[0:64, (u + 2) * BH:(u + 3) * BH],
                                         start=False, stop=True)
                    for i, u in enumerate((jj, jj + 1)):
                        p3a, p3b = ps3[2 * i], ps3[2 * i + 1]
                        fa = fa_pool.tile([128, BH], BF16, tag="fa")
                        nc.scalar.activation(fa, p3a, AF.Sign, bias=b3_s)
                        fb = fb_pool.tile([128, BH], BF16, tag="fb")
                        if u % 2 == 0:
                            # both halves +-1 via ACT, cheap 2x TT merge
                            nc.scalar.activation(fb, p3b, AF.Sign, bias=b3_s)
                            nc.vector.tensor_tensor(
                                s3t[:, u * BH:(u + 1) * BH], fa, fb,
                                op=ALU.max)
                        else:
                            nc.vector.tensor_scalar(
                                out=fb, in0=p3b, scalar1=th3_s, scalar2=0.5,
                                op0=ALU.is_ge, op1=ALU.subtract)
                            nc.vector.scalar_tensor_tensor(
                                s3t[:, u * BH:(u + 1) * BH], fa, 0.5, fb,
                                op0=ALU.mult, op1=ALU.max)

                # ---- L4: contract (ci,h), split by u-parity ---------------
                s3v = s3t.rearrange("p (u h b) -> p u h b", h=6, b=NB)
                ps4p = [psL4_pool.tile([128, 512], F32, tag="ps4",
                                        name=f"ps4_{p}") for p in range(2)]
                for h in range(6):
                    for par in range(2):
                        nc.tensor.matmul(
                            ps4p[par], W4_s[:, 128 * h:128 * (h + 1)],
                            s3v[:, par:16:2, h, :],
                            start=(h == 0), stop=(h == 5))
                for par in range(2):
                    nc.scalar.activation(
                        s4t[:, 512 * par:512 * (par + 1)], ps4p[par], AF.Sign,
                        bias=(b4e_s if par == 0 else b4o_s))

                # ---- fc ---------------------------------------------------
                psf = psL4_pool.tile([10, 64], F32, tag="ps4")
                for w in range(16):
                    blk = (w % 2) * 512 + (w // 2) * 64
                    nc.tensor.matmul(psf, Wf_s[:, 10 * w:10 * (w + 1)],
                                     s4t[:, blk:blk + 64],
                                     start=(w == 0), stop=(w == 15))
                outc = oc_pool.tile([10, NB], F32)
                nc.vector.tensor_scalar_add(outc, psf, bf_s)
                nc.sync.dma_start(
                    out=y_d[c * NB:(c + 1) * NB, :].rearrange("b j -> j b"),
                    in_=outc)

    nc.compile()
    return nc


_PROGRAM = None


def _get_program():
    global _PROGRAM
    if _PROGRAM is None:
        _PROGRAM = build_program()
    return _PROGRAM


def run(trace=False, **inputs):
    inputs = {k: np.asarray(v) for k, v in inputs.items()}
    consts = prepare_host_tensors(
        **{k: inputs[k] for k in
           ("w1", "b1", "w2", "b2", "w3", "b3", "w4", "b4",
            "g1", "be1", "m1", "v1", "g2", "be2", "m2", "v2",
            "g3", "be3", "m3", "v3", "g4", "be4", "m4", "v4", "wf", "bf")})
    x = inputs["x"].astype(np.float32)           # [8192, 1, 6, 128]
    nc = _get_program()
    in_maps = []
    for k in range(N_CORES):
        xc = x[k * B_CORE:(k + 1) * B_CORE, 0]               # [1024, 6, 128]
        # chunk-column order (c, h, b) so L4's rhs slices are contiguous
        xT = np.ascontiguousarray(
            xc.reshape(N_CHUNKS, NB, 6, 128).transpose(0, 2, 1, 3)
            .reshape(B_CORE * 6, 128).T)
        xh = xT.astype(np.float16)
        xl = (xT - xh.astype(np.float32)).astype(np.float16)
        m = {"xh": xh, "xl": xl}
        m.update(consts)
        in_maps.append(m)
    res = run_bass_kernel_spmd(nc, in_maps, list(range(N_CORES)), trace=trace)
    y = np.concatenate([r["y"] for r in res.results], axis=0)
    return y.astype(np.float32), res


def kernel(**inputs):
    y, _ = run(trace=False, **inputs)
    return y
